# revision 12
# baseline (speedup 1.0000x reference)
"""Trainium2 Bass kernel for nn_MultiHeadAttention_63814624084186.

Reference computation (per batch sample b, fully independent across b):
  x: [512, 4096]  (C channels x N=64*64 pixels)
  qkv = w_qkv @ x            -> q,k,v each [512, 4096] (8 heads x 64 dims)
  scores = (q_h @ k_h^T)/8   -> [64, 64] per head   (channel-attention)
  attn = softmax(scores, -1)
  out_h = attn_h @ v_h       -> [64, 4096]
  y = w_out @ out + b_out    -> [512, 4096]
  y = groupnorm(y over all C,N) * gamma + beta

Sharding: pure data-parallel over batch: 16 samples / 8 cores = 2 per core.

Algebraic restructure — all attention happens in channel space, so the
pixel-sized GEMMs can be collapsed:
  scores_h = q_h k_h^T = wq_h (x x^T) wk_h^T   -> Gram matrix G = x x^T
  y = w_out blockdiag(attn) wv x = W3 x        -> fold W3, never form v
Per-sample PE work: G (upper-tri, 41k cyc) + T1 = G wk^T (8k) + scores
(2k) + W2 fold (2k) + W3 fold (8k) + y = W3 x (66k) ~= 128k cycles vs
~246k for the direct q/k/v formulation.

Design notes:
  - G accumulates in 4 persistent PSUM banks (upper triangle only; the
    lower blocks are PE-transposed from the upper ones afterwards).
  - G/T1/W2/W3 all round to f16; resulting logit error ~0.01 and output
    error ~2e-3, well within tolerance.
  - GroupNorm: bn_stats per PSUM tile (bias folded into the cross-
    partition combine), cross-partition reduce via ones-matmul.
  - DMA descriptor generation (DIRECT2D, ~0.7us per call) serializes on
    the issuing engine, so transfers are split between the sync and
    gpsimd queues and all host-side layouts are per-partition
    contiguous (128 descriptors per DMA).
  - Batch 1's Gram is interleaved into batch 0's output GEMM; batch 0's
    epilogue hides under batch 1's output GEMM.  All 8 score
    accumulations share one PSUM bank so Gram(1) can grab banks the
    moment the G(0) copies drain.
"""

import numpy as np
from contextlib import ExitStack

import concourse.bass as bass
import concourse.tile as tile
from concourse import bacc, mybir
from concourse.bass_utils import run_bass_kernel_spmd

F32 = mybir.dt.float32
F16 = mybir.dt.float16
AX = mybir.AxisListType
ALU = mybir.AluOpType
ACTF = mybir.ActivationFunctionType

B = 16          # global batch
C = 512         # channels
N = 4096        # pixels (64*64)
HW_SIDE = 64
NCORES = 8
PB = B // NCORES  # batches per core
P = 128
KC = C // P     # 4 channel chunks
NB = 8          # pixel blocks of 512
NS = N // 512   # 8 pixel chunks of 512
NHP = 4         # head pairs
EPS = 1e-5


def build_nc():
    nc = bacc.Bacc("TRN2", target_bir_lowering=False, debug=False,
                   num_devices=NCORES)

    xt_d = nc.declare_dram_parameter("xt", [PB, NB, P, 4 * C], F16, isOutput=False)
    x_d = nc.declare_dram_parameter("x", [PB, 2, P, 4 * KC * 512], F16, isOutput=False)
    wall_d = nc.declare_dram_parameter("wall", [P, 4 * KC * C], F16,
                                       isOutput=False)
    eye_d = nc.declare_dram_parameter("eye", [P, P], F16, isOutput=False)
    bias_d = nc.declare_dram_parameter("bvec", [P, KC], F32, isOutput=False)
    gamma_d = nc.declare_dram_parameter("gamma", [P, KC], F32, isOutput=False)
    beta_d = nc.declare_dram_parameter("beta", [P, KC], F32, isOutput=False)
    out_d = nc.declare_dram_parameter("out", [PB, C, N], F16, isOutput=True)

    with tile.TileContext(nc) as tc, ExitStack() as ctx:
        consts = ctx.enter_context(tc.tile_pool(name="consts", bufs=1))
        xtpool = ctx.enter_context(tc.tile_pool(name="xtpool", bufs=6))
        xfpool = ctx.enter_context(tc.tile_pool(name="xfpool", bufs=2))
        gpool = ctx.enter_context(tc.tile_pool(name="gpool", bufs=1))
        t1pool = ctx.enter_context(tc.tile_pool(name="t1pool", bufs=1))
        w2pool = ctx.enter_context(tc.tile_pool(name="w2pool", bufs=1))
        w3pool = ctx.enter_context(tc.tile_pool(name="w3pool", bufs=1))
        ypool = ctx.enter_context(tc.tile_pool(name="ypool", bufs=4))
        attn = ctx.enter_context(tc.tile_pool(name="attn", bufs=2))
        attnt = ctx.enter_context(tc.tile_pool(name="attnt", bufs=4))
        stats = ctx.enter_context(tc.tile_pool(name="stats", bufs=2))
        psg = ctx.enter_context(tc.tile_pool(name="psg", bufs=4, space="PSUM"))
        psy = ctx.enter_context(tc.tile_pool(name="psy", bufs=3, space="PSUM"))
        pssc = ctx.enter_context(tc.tile_pool(name="pssc", bufs=1, space="PSUM"))

        def load_wall():
            t = consts.tile([P, 4, KC, C], F16, tag="wall")
            nc.sync.dma_start(
                out=t, in_=wall_d.ap().rearrange("p (w k c) -> p w k c",
                                                 w=4, k=KC))
            return t[:, 1], t[:, 0], t[:, 2], t[:, 3]  # wk, wq, wv, wo

        bias_sb = consts.tile([P, KC], F32, tag="bias")
        nc.gpsimd.dma_start(out=bias_sb, in_=bias_d[:, :])
        gamma_sb = consts.tile([P, KC], F32, tag="gamma")
        nc.gpsimd.dma_start(out=gamma_sb, in_=gamma_d[:, :])
        beta_sb = consts.tile([P, KC], F32, tag="beta")
        nc.gpsimd.dma_start(out=beta_sb, in_=beta_d[:, :])
        eye_sb = consts.tile([P, P], F16, tag="eye")
        nc.gpsimd.dma_start(out=eye_sb, in_=eye_d[:, :])

        eps_sb = consts.tile([1, 1], F32, tag="eps")
        nc.vector.memset(eps_sb, EPS)
        ones_col = consts.tile([P, 1], F32, tag="ones_col")
        nc.vector.memset(ones_col, 1.0)
        ones_row = consts.tile([1, P], F32, tag="ones_row")
        nc.vector.memset(ones_row, 1.0)

        # per-batch state carried between emission stages
        st_g = {}      # Gram PSUM banks
        st_gsb = {}    # G in SBUF (full, symmetrized)
        st_sc = {}     # score PSUM bank
        st_x = {}      # x [ch, pix] full tile
        st_w3 = {}     # folded W3^T
        st_y = {}
        st_stats = {}
        st_at = {}
        st_scale = {}

        def emit_G_setup(b):
            st_g[b] = [psg.tile([P, C - m * P], F32, tag="psg",
                                name=f"g_{b}_{m}") for m in range(KC)]

        st_xt = {}

        def emit_xt_dma(b, jb, split=False):
            """xt block DMA trigger (sync queue; order = consumption order)."""
            xt = xtpool.tile([P, 4, C], F16, tag="xt", name=f"xt_{b}_{jb}")
            st_xt[(b, jb)] = xt
            if split:
                for t in range(4):
                    nc.sync.dma_start(out=xt[:, t],
                                      in_=xt_d[b, jb, :, t * C:(t + 1) * C])
            else:
                nc.sync.dma_start(
                    out=xt, in_=xt_d[b, jb].rearrange("p (t c) -> p t c", t=4))

        def emit_G_mms(b, jb):
            """upper-tri Gram matmuls for one x^T block (512 pixels)."""
            gps = st_g[b]
            xt = st_xt[(b, jb)]
            for t in range(4):
                pix = jb * 4 + t
                for m in range(KC):
                    nc.tensor.matmul(
                        gps[m],
                        lhsT=xt[:, t, m * P:(m + 1) * P],
                        rhs=xt[:, t, m * P:C],
                        start=(pix == 0), stop=(pix == 31))

        def emit_x_half(b, h):
            """load 4 x [ch, pix] blocks for the output GEMM (contiguous)."""
            if b not in st_x:
                st_x[b] = xfpool.tile([P, NB, KC, 512], F16, tag="xf",
                                      name=f"xf_{b}")
            xf = st_x[b]
            nc.sync.dma_start(
                out=xf[:, 4 * h:4 * (h + 1)],
                in_=x_d[b, h].rearrange("p (j k n) -> p j k n", j=4, k=KC))

        def emit_M_copies(b):
            """PSUM -> SBUF eviction of the Gram upper triangle."""
            gps = st_g[b]
            g_sb = gpool.tile([P, KC, C], F16, tag="gsb", name=f"gsb_{b}")
            st_gsb[b] = g_sb
            for m in range(KC):
                eng = nc.scalar.copy if m % 2 == 0 else nc.vector.tensor_copy
                eng(out=g_sb[:, m, m * P:C], in_=gps[m])

        def emit_M_rest(b):
            """symmetrize G, T1 = G wk^T, per-head scores."""
            g_sb = st_gsb[b]
            # lower blocks via PE transpose of the upper ones
            for m in range(1, KC):
                for k in range(m):
                    pt = psy.tile([P, P], F16, tag="psy", padded_shape=[P, 512])
                    nc.tensor.transpose(pt, g_sb[:, k, m * P:(m + 1) * P],
                                        eye_sb)
                    nc.vector.tensor_copy(out=g_sb[:, m, k * P:(k + 1) * P],
                                          in_=pt)
            t1_sb = t1pool.tile([P, KC, C], F16, tag="t1", name=f"t1_{b}")
            for m in range(KC):
                ps = psy.tile([P, C], F32, tag="psy")
                for k in range(KC):
                    nc.tensor.matmul(
                        ps,
                        lhsT=g_sb[:, k, m * P:(m + 1) * P],
                        rhs=wk_sb[:, k, :],
                        start=(k == 0), stop=(k == KC - 1))
                nc.scalar.copy(out=t1_sb[:, m, :], in_=ps)
            sc_t = pssc.tile([P, NHP, 64], F32, tag="pssc", name=f"sc_{b}")
            st_sc[b] = sc_t
            for hp in range(NHP):
                cl0 = slice(hp * P, hp * P + 64)
                cl1 = slice(hp * P + 64, (hp + 1) * P)
                for k in range(KC):
                    nc.tensor.matmul(
                        sc_t[0:64, hp, :],
                        lhsT=wq_sb[:, k, cl0], rhs=t1_sb[:, k, cl0],
                        start=(k == 0), stop=(k == KC - 1),
                        skip_group_check=True)
                    nc.tensor.matmul(
                        sc_t[64:P, hp, :],
                        lhsT=wq_sb[:, k, cl1], rhs=t1_sb[:, k, cl1],
                        start=(k == 0), stop=(k == KC - 1),
                        skip_group_check=True)

        def emit_softmax(b):
            """softmax on the accumulated score blocks (all pairs batched)."""
            sc_t = st_sc[b]
            mx = attn.tile([P, NHP, 1], F32, tag="mx4")
            nc.vector.reduce_max(out=mx, in_=sc_t, axis=AX.X)
            d_all = attn.tile([P, NHP, 64], F32, tag="d_all")
            nc.vector.tensor_tensor(d_all, sc_t,
                                    mx.to_broadcast([P, NHP, 64]), ALU.subtract)
            e_all = attn.tile([P, NHP, 64], F32, tag="e_all")
            nc.scalar.activation(out=e_all, in_=d_all, func=ACTF.Exp,
                                 bias=0.0, scale=0.125)
            sm = attn.tile([P, NHP, 1], F32, tag="sm4")
            nc.vector.reduce_sum(out=sm, in_=e_all, axis=AX.X)
            rs = attn.tile([P, NHP, 1], F32, tag="rs4")
            nc.vector.reciprocal(out=rs, in_=sm)
            attnT_tiles = []
            for hp in range(NHP):
                at = attnt.tile([P, P], F16, tag="attnT", name=f"at_{b}_{hp}")
                nc.gpsimd.memset(at, 0.0)
                nc.vector.tensor_tensor(
                    at[0:64, 0:64], e_all[0:64, hp, :],
                    rs[0:64, hp, :].to_broadcast([64, 64]), ALU.mult)
                nc.vector.tensor_tensor(
                    at[64:P, 64:P], e_all[64:P, hp, :],
                    rs[64:P, hp, :].to_broadcast([64, 64]), ALU.mult)
                attnT_tiles.append(at)
            st_at[b] = attnT_tiles

        def emit_W3(b):
            """W2 = (blockdiag(A))^T @ woT, then W3^T = wv^T W2^T."""
            attnT_tiles = st_at[b]
            w2 = w2pool.tile([P, KC, C], F16, tag="w2", name=f"w2_{b}")
            for hp in range(NHP):
                at = attnT_tiles[hp]
                ps = psy.tile([P, C], F32, tag="psy")
                nc.tensor.matmul(ps, lhsT=at, rhs=wo_sb[:, hp, :],
                                 start=True, stop=True)
                (nc.scalar.copy if hp % 2 == 0
                 else nc.vector.tensor_copy)(out=w2[:, hp, :], in_=ps)
            w3 = w3pool.tile([P, KC, C], F16, tag="w3", name=f"w3_{b}")
            st_w3[b] = w3
            for m in range(KC):
                ps = psy.tile([P, C], F32, tag="psy")
                for kk in range(KC):
                    nc.tensor.matmul(
                        ps,
                        lhsT=wv_sb[:, kk, m * P:(m + 1) * P],
                        rhs=w2[:, kk, :],
                        start=(kk == 0), stop=(kk == KC - 1))
                (nc.scalar.copy if m % 2 == 0
                 else nc.vector.tensor_copy)(out=w3[:, m, :], in_=ps)

        def emit_By_setup(b):
            y_lo = ypool.tile([P, 2, N], F16, tag="y", name=f"ylo_{b}")
            y_hi = ypool.tile([P, 2, N], F16, tag="y", name=f"yhi_{b}")
            st_y[b] = (y_lo, y_hi)
            st_stats[b] = stats.tile([P, KC, 2], F32, tag="mv",
                                     name=f"mv_{b}")
            st_stats[(b, "raw")] = stats.tile([P, KC, NS, 6], F32,
                                              tag="bnstats", name=f"bst_{b}")

        def emit_By_unit(b, m, ns):
            """output GEMM y[m-chunk, ns-block] = W3[m-chunk, :] @ x + stats."""
            w3 = st_w3[b]
            xf = st_x[b]
            y_lo, y_hi = st_y[b]
            st = st_stats[(b, "raw")]
            yt = y_lo if m < 2 else y_hi
            mi = m % 2
            ps = psy.tile([P, 512], F32, tag="psy")
            for k in range(KC):
                nc.tensor.matmul(
                    ps,
                    lhsT=w3[:, k, m * P:(m + 1) * P],
                    rhs=xf[:, ns, k, :],
                    start=(k == 0), stop=(k == KC - 1))
            nc.vector.bn_stats(out=st[:, m, ns, :], in_=ps)
            nc.scalar.add(out=yt[:, mi, ns * 512:(ns + 1) * 512],
                          in_=ps, add=bias_sb[:, m:m + 1])
            if ns == NS - 1:
                nc.vector.bn_aggr(out=st_stats[b][:, m, :], in_=st[:, m])

        def emit_tail_stats(b):
            """global mean/var combine."""
            mv = st_stats[b]
            # S[p, stat, m]: 0 = mean+bias, 1 = var, 2 = (mean+bias)^2
            s_t = stats.tile([P, 3, KC], F32, tag="s_t")
            nc.vector.tensor_add(s_t[:, 0, :], mv[:, :, 0], bias_sb)
            nc.vector.tensor_copy(out=s_t[:, 1, :], in_=mv[:, :, 1])
            nc.vector.tensor_mul(s_t[:, 2, :], s_t[:, 0, :], s_t[:, 0, :])
            pstat = psy.tile([1, 3, KC], F32, tag="psy")
            nc.tensor.matmul(pstat, lhsT=ones_col, rhs=s_t,
                             start=True, stop=True)
            red = stats.tile([1, 3], F32, tag="red")
            nc.vector.reduce_sum(out=red, in_=pstat, axis=AX.X)
            e3 = stats.tile([1, 3], F32, tag="e3")
            nc.vector.tensor_scalar_mul(e3, red, 1.0 / C)
            m2 = stats.tile([1, 1], F32, tag="m2")
            nc.vector.tensor_mul(m2, e3[:, 0:1], e3[:, 0:1])
            var = stats.tile([1, 1], F32, tag="var")
            nc.vector.tensor_add(var, e3[:, 1:2], e3[:, 2:3])
            nc.vector.tensor_sub(var, var, m2)
            sc2 = stats.tile([1, 2], F32, tag="sc2")
            nc.vector.tensor_copy(out=sc2[:, 0:1], in_=e3[:, 0:1])
            std = stats.tile([1, 1], F32, tag="std")
            nc.scalar.activation(out=std, in_=var, func=ACTF.Sqrt,
                                 bias=eps_sb, scale=1.0)
            nc.vector.reciprocal(out=sc2[:, 1:2], in_=std)
            bc_ps = psy.tile([P, 2], F32, tag="psy")
            nc.tensor.matmul(bc_ps, lhsT=ones_row, rhs=sc2,
                             start=True, stop=True)
            # s = gamma * rstd ; t = beta - mean_total * s
            s_ch = stats.tile([P, KC], F32, tag="s_ch")
            nc.vector.tensor_scalar_mul(s_ch, gamma_sb, bc_ps[:, 1:2])
            t_ch = stats.tile([P, KC], F32, tag="t_ch")
            nc.vector.tensor_scalar_mul(t_ch, s_ch, bc_ps[:, 0:1])
            nc.vector.tensor_sub(t_ch, beta_sb, t_ch)
            st_scale[b] = (s_ch, t_ch)

        APPLY_SPLIT = 3 * N // 4   # vector is ~3x faster than scalar here

        def emit_apply_slice(b, m, h):
            """normalization apply for one (chunk, section) + writeout."""
            y_lo, y_hi = st_y[b]
            s_ch, t_ch = st_scale[b]
            yt = y_lo if m < 2 else y_hi
            mi = m % 2
            sl = slice(0, APPLY_SPLIT) if h == 0 else slice(APPLY_SPLIT, N)
            if h == 0:
                nc.vector.tensor_scalar(
                    out=yt[:, mi, sl], in0=yt[:, mi, sl],
                    scalar1=s_ch[:, m:m + 1], scalar2=t_ch[:, m:m + 1],
                    op0=ALU.mult, op1=ALU.add)
            else:
                nc.scalar.activation(
                    out=yt[:, mi, sl], in_=yt[:, mi, sl],
                    func=ACTF.Identity,
                    bias=t_ch[:, m:m + 1], scale=s_ch[:, m:m + 1])
            eng = nc.sync if (m + h) % 2 == 0 else nc.gpsimd
            eng.dma_start(out=out_d[b, m * P:(m + 1) * P, sl],
                          in_=yt[:, mi, sl])

        def emit_tail_apply(b):
            for m in range(KC):
                for h in range(2):
                    emit_apply_slice(b, m, h)

        # ---- emission schedule ----
        # sync-queue DMA triggers in exact consumption order: the xt tile
        # slot semaphores pace the whole input stream, and the per-engine
        # descriptor FIFOs then deliver transfers in the same order.
        emit_G_setup(0)
        emit_xt_dma(0, 0, split=True)
        for jb in range(1, NB):
            emit_xt_dma(0, jb)
        wk_sb, wq_sb, wv_sb, wo_sb = load_wall()
        emit_G_setup(1)
        emit_xt_dma(1, 0)
        emit_xt_dma(1, 1)
        emit_x_half(0, 0)
        emit_x_half(0, 1)
        for jb in range(NB):
            emit_G_mms(0, jb)
        emit_M_copies(0)
        emit_M_rest(0)
        emit_xt_dma(1, 2)
        emit_xt_dma(1, 3)
        emit_G_mms(1, 0)
        emit_G_mms(1, 1)
        emit_G_mms(1, 2)
        emit_G_mms(1, 3)
        emit_softmax(0)
        emit_W3(0)
        emit_By_setup(0)
        # interleave batch 0's output GEMM with batch 1's Gram; the last
        # four units are held back to cover batch 1's M-phase latency
        nxt = 4
        u = 0
        held = [(3, ns) for ns in range(4, NS)]
        for m in range(KC):
            for ns in range(NS):
                if (m, ns) in held:
                    continue
                emit_By_unit(0, m, ns)
                u += 1
                if u % 6 == 0 and nxt < NB:
                    emit_xt_dma(1, nxt)
                    emit_G_mms(1, nxt)
                    nxt += 1
                if u == 18:
                    emit_x_half(1, 0)
                if u == 22:
                    emit_x_half(1, 1)
                if u == 25:
                    emit_M_copies(1)
                if u == 26:
                    emit_M_rest(1)
        emit_softmax(1)
        # pre-warm the Sqrt activation table while the PE is busy
        warm = stats.tile([1, 1], F32, tag="warm")
        nc.scalar.sqrt(out=warm, in_=eps_sb)
        for hu in held:
            emit_By_unit(0, *hu)
        emit_W3(1)
        emit_By_setup(1)
        ap_i = 0
        v = 0
        for m in range(KC):
            for ns in range(NS):
                emit_By_unit(1, m, ns)
                v += 1
                if v == 2:
                    emit_tail_stats(0)
                if v >= 6 and v % 3 == 0 and ap_i < 2 * KC:
                    emit_apply_slice(0, ap_i // 2, ap_i % 2)
                    ap_i += 1
        emit_tail_stats(1)
        emit_tail_apply(1)

    nc.finalize()
    return nc


_NC_CACHE = {}


def _get_nc():
    if "nc" not in _NC_CACHE:
        _NC_CACHE["nc"] = build_nc()
    return _NC_CACHE["nc"]


def _prep_w(w):
    # [C_in, C_out] -> [128, KC, C_out] fp16 with c_in = k*128 + p
    return np.ascontiguousarray(
        w.reshape(KC, P, C).transpose(1, 0, 2).astype(np.float16))


def _prep_vec(v):
    # [C] -> [128, KC] with c = k*128 + p
    return np.ascontiguousarray(v.reshape(KC, P).T)


def _prep_x(x):
    # [B, C, N] -> [B, 2, P, 4*KC*512] fp16: half h holds pixel blocks
    # 4h..4h+3; per-partition payload (j_local, k, n) is contiguous.
    nb = np.asarray(x).shape[0]
    xr = np.asarray(x, dtype=np.float32).reshape(nb, KC, P, 2, 4, 512)
    return np.ascontiguousarray(
        xr.transpose(0, 3, 2, 4, 1, 5).astype(np.float16)).reshape(
        nb, 2, P, 4 * KC * 512)


def _prep_xt(x):
    # [B, C, N] -> [B, NB, P, 4*C] fp16: xt[b, jb, p, t*C+c] =
    #   x[b, c, (jb*4+t)*128 + p]
    nb = np.asarray(x).shape[0]
    xr = np.asarray(x, dtype=np.float32).reshape(nb, C, NB, 4, P)
    return np.ascontiguousarray(
        xr.transpose(0, 2, 4, 3, 1).astype(np.float16)).reshape(
        nb, NB, P, 4 * C)


def _make_in_maps(x, w_qkv, w_out, b_out, gamma, beta):
    x = np.asarray(x)
    xr = _prep_x(x)
    xtr = _prep_xt(x)
    w_qkv = np.asarray(w_qkv, dtype=np.float32)
    wq = _prep_w(np.ascontiguousarray(w_qkv[0:C].T))
    wk = _prep_w(np.ascontiguousarray(w_qkv[C:2 * C].T))
    # wv in natural orientation: [v-ch, in-ch] chunked along v-ch
    wv = _prep_w(np.ascontiguousarray(w_qkv[2 * C:3 * C]))
    wo = _prep_w(np.ascontiguousarray(np.asarray(w_out, dtype=np.float32).T))
    wall = np.ascontiguousarray(
        np.stack([wq, wk, wv, wo], axis=1)).reshape(P, 4 * KC * C)
    eye = np.eye(P, dtype=np.float16)
    bvec = _prep_vec(np.asarray(b_out, dtype=np.float32))
    gam = _prep_vec(np.asarray(gamma, dtype=np.float32))
    bet = _prep_vec(np.asarray(beta, dtype=np.float32))
    return [
        dict(x=np.ascontiguousarray(xr[c * PB:(c + 1) * PB]),
             xt=np.ascontiguousarray(xtr[c * PB:(c + 1) * PB]),
             wall=wall, eye=eye,
             bvec=bvec, gamma=gam, beta=bet)
        for c in range(NCORES)
    ]


def _run(inputs, trace=False, trace_kwargs=None):
    nc = _get_nc()
    in_maps = _make_in_maps(**inputs)
    res = run_bass_kernel_spmd(nc, in_maps, core_ids=list(range(NCORES)),
                               trace=trace, **(trace_kwargs or {}))
    out = np.concatenate([res.results[c]["out"].astype(np.float32)
                          for c in range(NCORES)], axis=0)
    return out.reshape(B, C, HW_SIDE, HW_SIDE), res


def kernel(x, w_qkv, w_out, b_out, gamma, beta):
    inputs = dict(x=x, w_qkv=w_qkv, w_out=w_out, b_out=b_out,
                  gamma=gamma, beta=beta)
    try:
        out, _ = _run(inputs)
    except Exception:
        # transient device errors (e.g. NRT_EXEC_UNIT_UNRECOVERABLE) have
        # been observed once across many runs; one retry recovers.
        out, _ = _run(inputs)
    return out


# revision 14
# speedup vs baseline: 1.0343x; 1.0343x over previous
"""Trainium2 Bass kernel for nn_MultiHeadAttention_63814624084186.

Reference computation (per batch sample b, fully independent across b):
  x: [512, 4096]  (C channels x N=64*64 pixels)
  qkv = w_qkv @ x            -> q,k,v each [512, 4096] (8 heads x 64 dims)
  scores = (q_h @ k_h^T)/8   -> [64, 64] per head   (channel-attention)
  attn = softmax(scores, -1)
  out_h = attn_h @ v_h       -> [64, 4096]
  y = w_out @ out + b_out    -> [512, 4096]
  y = groupnorm(y over all C,N) * gamma + beta

Sharding: pure data-parallel over batch: 16 samples / 8 cores = 2 per core.

Algebraic restructure — all attention happens in channel space, so the
pixel-sized GEMMs can be collapsed:
  scores_h = q_h k_h^T = wq_h (x x^T) wk_h^T   -> Gram matrix G = x x^T
  y = w_out blockdiag(attn) wv x = W3 x        -> fold W3, never form v
Per-sample PE work: G (upper-tri, 41k cyc) + T1 = G wk^T (8k) + scores
(2k) + W2 fold (2k) + W3 fold (8k) + y = W3 x (66k) ~= 128k cycles vs
~246k for the direct q/k/v formulation.

Design notes:
  - G accumulates in 4 persistent PSUM banks (upper triangle only; the
    lower blocks are PE-transposed from the upper ones afterwards).
  - G/T1/W2/W3 all round to f16; resulting logit error ~0.01 and output
    error ~2e-3, well within tolerance.
  - GroupNorm: bn_stats per PSUM tile (bias folded into the cross-
    partition combine), cross-partition reduce via ones-matmul.
  - DMA descriptor generation (DIRECT2D, ~0.7us per call) serializes on
    the issuing engine, so transfers are split between the sync and
    gpsimd queues and all host-side layouts are per-partition
    contiguous (128 descriptors per DMA).
  - Batch 1's Gram is interleaved into batch 0's output GEMM; batch 0's
    epilogue hides under batch 1's output GEMM.  All 8 score
    accumulations share one PSUM bank so Gram(1) can grab banks the
    moment the G(0) copies drain.
"""

import numpy as np
from contextlib import ExitStack

import concourse.bass as bass
import concourse.tile as tile
from concourse import bacc, mybir
from concourse.bass_utils import run_bass_kernel_spmd

F32 = mybir.dt.float32
F16 = mybir.dt.float16
AX = mybir.AxisListType
ALU = mybir.AluOpType
ACTF = mybir.ActivationFunctionType

B = 16          # global batch
C = 512         # channels
N = 4096        # pixels (64*64)
HW_SIDE = 64
NCORES = 8
PB = B // NCORES  # batches per core
P = 128
KC = C // P     # 4 channel chunks
NB = 8          # pixel blocks of 512
NS = N // 512   # 8 pixel chunks of 512
NHP = 4         # head pairs
EPS = 1e-5


def build_nc():
    nc = bacc.Bacc("TRN2", target_bir_lowering=False, debug=False,
                   num_devices=NCORES)

    xt_d = nc.declare_dram_parameter("xt", [PB, NB, P, 4 * C], F16, isOutput=False)
    x_d = nc.declare_dram_parameter("x", [PB, 2, P, 4 * KC * 512], F16, isOutput=False)
    wall_d = nc.declare_dram_parameter("wall", [P, 4 * KC * C], F16,
                                       isOutput=False)
    eye_d = nc.declare_dram_parameter("eye", [P, P], F16, isOutput=False)
    bias_d = nc.declare_dram_parameter("bvec", [P, KC], F32, isOutput=False)
    gamma_d = nc.declare_dram_parameter("gamma", [P, KC], F32, isOutput=False)
    beta_d = nc.declare_dram_parameter("beta", [P, KC], F32, isOutput=False)
    out_d = nc.declare_dram_parameter("out", [PB, C, N], F16, isOutput=True)

    with tile.TileContext(nc) as tc, ExitStack() as ctx:
        consts = ctx.enter_context(tc.tile_pool(name="consts", bufs=1))
        xtpool = ctx.enter_context(tc.tile_pool(name="xtpool", bufs=6))
        xfpool = ctx.enter_context(tc.tile_pool(name="xfpool", bufs=2))
        gpool = ctx.enter_context(tc.tile_pool(name="gpool", bufs=1))
        t1pool = ctx.enter_context(tc.tile_pool(name="t1pool", bufs=1))
        w2pool = ctx.enter_context(tc.tile_pool(name="w2pool", bufs=1))
        w3pool = ctx.enter_context(tc.tile_pool(name="w3pool", bufs=1))
        ypool = ctx.enter_context(tc.tile_pool(name="ypool", bufs=4))
        attn = ctx.enter_context(tc.tile_pool(name="attn", bufs=2))
        attnt = ctx.enter_context(tc.tile_pool(name="attnt", bufs=4))
        stats = ctx.enter_context(tc.tile_pool(name="stats", bufs=2))
        psg = ctx.enter_context(tc.tile_pool(name="psg", bufs=4, space="PSUM"))
        psy = ctx.enter_context(tc.tile_pool(name="psy", bufs=3, space="PSUM"))
        pssc = ctx.enter_context(tc.tile_pool(name="pssc", bufs=1, space="PSUM"))

        def load_wall():
            t = consts.tile([P, 4, KC, C], F16, tag="wall")
            nc.sync.dma_start(
                out=t, in_=wall_d.ap().rearrange("p (w k c) -> p w k c",
                                                 w=4, k=KC))
            return t[:, 1], t[:, 0], t[:, 2], t[:, 3]  # wk, wq, wv, wo

        bias_sb = consts.tile([P, KC], F32, tag="bias")
        nc.gpsimd.dma_start(out=bias_sb, in_=bias_d[:, :])
        gamma_sb = consts.tile([P, KC], F32, tag="gamma")
        nc.gpsimd.dma_start(out=gamma_sb, in_=gamma_d[:, :])
        beta_sb = consts.tile([P, KC], F32, tag="beta")
        nc.gpsimd.dma_start(out=beta_sb, in_=beta_d[:, :])
        eye_sb = consts.tile([P, P], F16, tag="eye")
        nc.gpsimd.dma_start(out=eye_sb, in_=eye_d[:, :])

        eps_sb = consts.tile([1, 1], F32, tag="eps")
        nc.vector.memset(eps_sb, EPS)
        ones_col = consts.tile([P, 1], F32, tag="ones_col")
        nc.vector.memset(ones_col, 1.0)
        ones_row = consts.tile([1, P], F32, tag="ones_row")
        nc.vector.memset(ones_row, 1.0)

        # per-batch state carried between emission stages
        st_g = {}      # Gram PSUM banks
        st_gsb = {}    # G in SBUF (full, symmetrized)
        st_sc = {}     # score PSUM bank
        st_x = {}      # x [ch, pix] full tile
        st_w3 = {}     # folded W3^T
        st_y = {}
        st_stats = {}
        st_at = {}
        st_scale = {}

        def emit_G_setup(b):
            st_g[b] = [psg.tile([P, C - m * P], F32, tag="psg",
                                name=f"g_{b}_{m}") for m in range(KC)]

        st_xt = {}

        def emit_xt_dma(b, jb, split=False):
            """xt block DMA trigger (sync queue; order = consumption order)."""
            xt = xtpool.tile([P, 4, C], F16, tag="xt", name=f"xt_{b}_{jb}")
            st_xt[(b, jb)] = xt
            if split:
                for t in range(4):
                    nc.sync.dma_start(out=xt[:, t],
                                      in_=xt_d[b, jb, :, t * C:(t + 1) * C])
            else:
                nc.sync.dma_start(
                    out=xt, in_=xt_d[b, jb].rearrange("p (t c) -> p t c", t=4))

        def emit_G_mms(b, jb):
            """upper-tri Gram matmuls for one x^T block (512 pixels)."""
            gps = st_g[b]
            xt = st_xt[(b, jb)]
            for t in range(4):
                pix = jb * 4 + t
                for m in range(KC):
                    nc.tensor.matmul(
                        gps[m],
                        lhsT=xt[:, t, m * P:(m + 1) * P],
                        rhs=xt[:, t, m * P:C],
                        start=(pix == 0), stop=(pix == 31))

        def emit_x_half(b, h):
            """load 4 x [ch, pix] blocks for the output GEMM (contiguous)."""
            if b not in st_x:
                st_x[b] = xfpool.tile([P, NB, KC, 512], F16, tag="xf",
                                      name=f"xf_{b}")
            xf = st_x[b]
            nc.sync.dma_start(
                out=xf[:, 4 * h:4 * (h + 1)],
                in_=x_d[b, h].rearrange("p (j k n) -> p j k n", j=4, k=KC))

        def emit_M_copies(b):
            """PSUM -> SBUF eviction of the Gram upper triangle."""
            gps = st_g[b]
            g_sb = gpool.tile([P, KC, C], F16, tag="gsb", name=f"gsb_{b}")
            st_gsb[b] = g_sb
            for m in range(KC):
                eng = nc.scalar.copy if m % 2 == 0 else nc.vector.tensor_copy
                eng(out=g_sb[:, m, m * P:C], in_=gps[m])

        def emit_M_rest(b):
            """symmetrize G, T1 = G wk^T, per-head scores."""
            g_sb = st_gsb[b]
            # lower blocks via PE transpose of the upper ones
            for m in range(1, KC):
                for k in range(m):
                    pt = psy.tile([P, P], F16, tag="psy", padded_shape=[P, 512])
                    nc.tensor.transpose(pt, g_sb[:, k, m * P:(m + 1) * P],
                                        eye_sb)
                    nc.vector.tensor_copy(out=g_sb[:, m, k * P:(k + 1) * P],
                                          in_=pt)
            t1_sb = t1pool.tile([P, KC, C], F16, tag="t1", name=f"t1_{b}")
            for m in range(KC):
                ps = psy.tile([P, C], F32, tag="psy")
                for k in range(KC):
                    nc.tensor.matmul(
                        ps,
                        lhsT=g_sb[:, k, m * P:(m + 1) * P],
                        rhs=wk_sb[:, k, :],
                        start=(k == 0), stop=(k == KC - 1))
                nc.scalar.copy(out=t1_sb[:, m, :], in_=ps)
            sc_t = pssc.tile([P, NHP, 64], F32, tag="pssc", name=f"sc_{b}")
            st_sc[b] = sc_t
            for hp in range(NHP):
                cl0 = slice(hp * P, hp * P + 64)
                cl1 = slice(hp * P + 64, (hp + 1) * P)
                for k in range(KC):
                    nc.tensor.matmul(
                        sc_t[0:64, hp, :],
                        lhsT=wq_sb[:, k, cl0], rhs=t1_sb[:, k, cl0],
                        start=(k == 0), stop=(k == KC - 1),
                        skip_group_check=True)
                    nc.tensor.matmul(
                        sc_t[64:P, hp, :],
                        lhsT=wq_sb[:, k, cl1], rhs=t1_sb[:, k, cl1],
                        start=(k == 0), stop=(k == KC - 1),
                        skip_group_check=True)

        def emit_softmax(b):
            """softmax on the accumulated score blocks (all pairs batched)."""
            sc_t = st_sc[b]
            mx = attn.tile([P, NHP, 1], F32, tag="mx4")
            nc.vector.reduce_max(out=mx, in_=sc_t, axis=AX.X)
            d_all = attn.tile([P, NHP, 64], F32, tag="d_all")
            nc.vector.tensor_tensor(d_all, sc_t,
                                    mx.to_broadcast([P, NHP, 64]), ALU.subtract)
            e_all = attn.tile([P, NHP, 64], F32, tag="e_all")
            nc.scalar.activation(out=e_all, in_=d_all, func=ACTF.Exp,
                                 bias=0.0, scale=0.125)
            sm = attn.tile([P, NHP, 1], F32, tag="sm4")
            nc.vector.reduce_sum(out=sm, in_=e_all, axis=AX.X)
            rs = attn.tile([P, NHP, 1], F32, tag="rs4")
            nc.vector.reciprocal(out=rs, in_=sm)
            attnT_tiles = []
            for hp in range(NHP):
                at = attnt.tile([P, P], F16, tag="attnT", name=f"at_{b}_{hp}")
                nc.gpsimd.memset(at, 0.0)
                nc.vector.tensor_tensor(
                    at[0:64, 0:64], e_all[0:64, hp, :],
                    rs[0:64, hp, :].to_broadcast([64, 64]), ALU.mult)
                nc.vector.tensor_tensor(
                    at[64:P, 64:P], e_all[64:P, hp, :],
                    rs[64:P, hp, :].to_broadcast([64, 64]), ALU.mult)
                attnT_tiles.append(at)
            st_at[b] = attnT_tiles

        def emit_W3(b):
            """W2 = (blockdiag(A))^T @ woT, then W3^T = wv^T W2^T."""
            attnT_tiles = st_at[b]
            w2 = w2pool.tile([P, KC, C], F16, tag="w2", name=f"w2_{b}")
            for hp in range(NHP):
                at = attnT_tiles[hp]
                ps = psy.tile([P, C], F32, tag="psy")
                nc.tensor.matmul(ps, lhsT=at, rhs=wo_sb[:, hp, :],
                                 start=True, stop=True)
                (nc.scalar.copy if hp % 2 == 0
                 else nc.vector.tensor_copy)(out=w2[:, hp, :], in_=ps)
            w3 = w3pool.tile([P, KC, C], F16, tag="w3", name=f"w3_{b}")
            st_w3[b] = w3
            for m in range(KC):
                ps = psy.tile([P, C], F32, tag="psy")
                for kk in range(KC):
                    nc.tensor.matmul(
                        ps,
                        lhsT=wv_sb[:, kk, m * P:(m + 1) * P],
                        rhs=w2[:, kk, :],
                        start=(kk == 0), stop=(kk == KC - 1))
                (nc.scalar.copy if m % 2 == 0
                 else nc.vector.tensor_copy)(out=w3[:, m, :], in_=ps)

        def emit_By_setup(b):
            y_lo = ypool.tile([P, 2, N], F16, tag="y", name=f"ylo_{b}")
            y_hi = ypool.tile([P, 2, N], F16, tag="y", name=f"yhi_{b}")
            st_y[b] = (y_lo, y_hi)
            st_stats[b] = stats.tile([P, KC, 2], F32, tag="mv",
                                     name=f"mv_{b}")
            st_stats[(b, "raw")] = stats.tile([P, KC, NS, 6], F32,
                                              tag="bnstats", name=f"bst_{b}")

        def emit_By_unit(b, m, ns, pool=None):
            """output GEMM y[m-chunk, ns-block] = W3[m-chunk, :] @ x + stats."""
            w3 = st_w3[b]
            xf = st_x[b]
            y_lo, y_hi = st_y[b]
            st = st_stats[(b, "raw")]
            yt = y_lo if m < 2 else y_hi
            mi = m % 2
            if pool is None:
                ps = psy.tile([P, 512], F32, tag="psy")
            else:
                ps = pool.tile([P, 512], F32, tag="psg", name=f"byg_{b}_{m}_{ns}")
            for k in range(KC):
                nc.tensor.matmul(
                    ps,
                    lhsT=w3[:, k, m * P:(m + 1) * P],
                    rhs=xf[:, ns, k, :],
                    start=(k == 0), stop=(k == KC - 1))
            nc.vector.bn_stats(out=st[:, m, ns, :], in_=ps)
            nc.scalar.add(out=yt[:, mi, ns * 512:(ns + 1) * 512],
                          in_=ps, add=bias_sb[:, m:m + 1])
            if ns == NS - 1:
                nc.vector.bn_aggr(out=st_stats[b][:, m, :], in_=st[:, m])

        def emit_tail_stats(b):
            """global mean/var combine."""
            mv = st_stats[b]
            # S[p, stat, m]: 0 = mean+bias, 1 = var, 2 = (mean+bias)^2
            s_t = stats.tile([P, 3, KC], F32, tag="s_t")
            nc.vector.tensor_add(s_t[:, 0, :], mv[:, :, 0], bias_sb)
            nc.vector.tensor_copy(out=s_t[:, 1, :], in_=mv[:, :, 1])
            nc.vector.tensor_mul(s_t[:, 2, :], s_t[:, 0, :], s_t[:, 0, :])
            pstat = psy.tile([1, 3, KC], F32, tag="psy")
            nc.tensor.matmul(pstat, lhsT=ones_col, rhs=s_t,
                             start=True, stop=True)
            red = stats.tile([1, 3], F32, tag="red")
            nc.vector.reduce_sum(out=red, in_=pstat, axis=AX.X)
            e3 = stats.tile([1, 3], F32, tag="e3")
            nc.vector.tensor_scalar_mul(e3, red, 1.0 / C)
            m2 = stats.tile([1, 1], F32, tag="m2")
            nc.vector.tensor_mul(m2, e3[:, 0:1], e3[:, 0:1])
            var = stats.tile([1, 1], F32, tag="var")
            nc.vector.tensor_add(var, e3[:, 1:2], e3[:, 2:3])
            nc.vector.tensor_sub(var, var, m2)
            sc2 = stats.tile([1, 2], F32, tag="sc2")
            nc.vector.tensor_copy(out=sc2[:, 0:1], in_=e3[:, 0:1])
            std = stats.tile([1, 1], F32, tag="std")
            nc.scalar.activation(out=std, in_=var, func=ACTF.Sqrt,
                                 bias=eps_sb, scale=1.0)
            nc.vector.reciprocal(out=sc2[:, 1:2], in_=std)
            bc_ps = psy.tile([P, 2], F32, tag="psy")
            nc.tensor.matmul(bc_ps, lhsT=ones_row, rhs=sc2,
                             start=True, stop=True)
            # s = gamma * rstd ; t = beta - mean_total * s
            s_ch = stats.tile([P, KC], F32, tag="s_ch")
            nc.vector.tensor_scalar_mul(s_ch, gamma_sb, bc_ps[:, 1:2])
            t_ch = stats.tile([P, KC], F32, tag="t_ch")
            nc.vector.tensor_scalar_mul(t_ch, s_ch, bc_ps[:, 0:1])
            nc.vector.tensor_sub(t_ch, beta_sb, t_ch)
            st_scale[b] = (s_ch, t_ch)

        APPLY_SPLIT = 3 * N // 4   # vector is ~3x faster than scalar here

        def emit_apply_slice(b, m, h):
            """normalization apply for one (chunk, section) + writeout."""
            y_lo, y_hi = st_y[b]
            s_ch, t_ch = st_scale[b]
            yt = y_lo if m < 2 else y_hi
            mi = m % 2
            sl = slice(0, APPLY_SPLIT) if h == 0 else slice(APPLY_SPLIT, N)
            if h == 0:
                nc.vector.tensor_scalar(
                    out=yt[:, mi, sl], in0=yt[:, mi, sl],
                    scalar1=s_ch[:, m:m + 1], scalar2=t_ch[:, m:m + 1],
                    op0=ALU.mult, op1=ALU.add)
            else:
                nc.scalar.activation(
                    out=yt[:, mi, sl], in_=yt[:, mi, sl],
                    func=ACTF.Identity,
                    bias=t_ch[:, m:m + 1], scale=s_ch[:, m:m + 1])
            eng = nc.sync if (m + h) % 2 == 0 else nc.gpsimd
            eng.dma_start(out=out_d[b, m * P:(m + 1) * P, sl],
                          in_=yt[:, mi, sl])

        def emit_tail_apply(b):
            for m in range(KC):
                for h in range(2):
                    emit_apply_slice(b, m, h)

        # ---- emission schedule ----
        # sync-queue DMA triggers in exact consumption order: the xt tile
        # slot semaphores pace the whole input stream, and the per-engine
        # descriptor FIFOs then deliver transfers in the same order.
        emit_G_setup(0)
        emit_xt_dma(0, 0, split=True)
        for jb in range(1, NB):
            emit_xt_dma(0, jb)
        wk_sb, wq_sb, wv_sb, wo_sb = load_wall()
        emit_G_setup(1)
        emit_xt_dma(1, 0)
        emit_xt_dma(1, 1)
        emit_x_half(0, 0)
        emit_x_half(0, 1)
        for jb in range(NB):
            emit_G_mms(0, jb)
        emit_M_copies(0)
        emit_M_rest(0)
        emit_xt_dma(1, 2)
        emit_xt_dma(1, 3)
        emit_G_mms(1, 0)
        emit_G_mms(1, 1)
        emit_G_mms(1, 2)
        emit_G_mms(1, 3)
        emit_softmax(0)
        emit_W3(0)
        emit_By_setup(0)
        # interleave batch 0's output GEMM with batch 1's Gram; the last
        # four units are held back to cover batch 1's M-phase latency
        nxt = 4
        u = 0
        held = [(3, ns) for ns in range(4, NS)]
        for m in range(KC):
            for ns in range(NS):
                if (m, ns) in held:
                    continue
                emit_By_unit(0, m, ns)
                u += 1
                if u % 6 == 0 and nxt < NB:
                    emit_xt_dma(1, nxt)
                    emit_G_mms(1, nxt)
                    nxt += 1
                if u == 18:
                    emit_x_half(1, 0)
                if u == 22:
                    emit_x_half(1, 1)
                if u == 25:
                    emit_M_copies(1)
                if u == 26:
                    emit_M_rest(1)
        emit_softmax(1)
        # pre-warm the Sqrt activation table while the PE is busy
        warm = stats.tile([1, 1], F32, tag="warm")
        nc.scalar.sqrt(out=warm, in_=eps_sb)
        for hu in held:
            emit_By_unit(0, *hu)
        emit_W3(1)
        emit_By_setup(1)
        ap_i = 0
        v = 0
        for m in range(KC):
            for ns in range(NS):
                emit_By_unit(1, m, ns, pool=psg if ns % 2 == 0 else None)
                v += 1
                if v == 2:
                    emit_tail_stats(0)
                if v >= 6 and v % 3 == 0 and ap_i < 2 * KC:
                    emit_apply_slice(0, ap_i // 2, ap_i % 2)
                    ap_i += 1
        emit_tail_stats(1)
        emit_tail_apply(1)

    nc.finalize()
    return nc


_NC_CACHE = {}


def _get_nc():
    if "nc" not in _NC_CACHE:
        _NC_CACHE["nc"] = build_nc()
    return _NC_CACHE["nc"]


def _prep_w(w):
    # [C_in, C_out] -> [128, KC, C_out] fp16 with c_in = k*128 + p
    return np.ascontiguousarray(
        w.reshape(KC, P, C).transpose(1, 0, 2).astype(np.float16))


def _prep_vec(v):
    # [C] -> [128, KC] with c = k*128 + p
    return np.ascontiguousarray(v.reshape(KC, P).T)


def _prep_x(x):
    # [B, C, N] -> [B, 2, P, 4*KC*512] fp16: half h holds pixel blocks
    # 4h..4h+3; per-partition payload (j_local, k, n) is contiguous.
    nb = np.asarray(x).shape[0]
    xr = np.asarray(x, dtype=np.float32).reshape(nb, KC, P, 2, 4, 512)
    return np.ascontiguousarray(
        xr.transpose(0, 3, 2, 4, 1, 5).astype(np.float16)).reshape(
        nb, 2, P, 4 * KC * 512)


def _prep_xt(x):
    # [B, C, N] -> [B, NB, P, 4*C] fp16: xt[b, jb, p, t*C+c] =
    #   x[b, c, (jb*4+t)*128 + p]
    nb = np.asarray(x).shape[0]
    xr = np.asarray(x, dtype=np.float32).reshape(nb, C, NB, 4, P)
    return np.ascontiguousarray(
        xr.transpose(0, 2, 4, 3, 1).astype(np.float16)).reshape(
        nb, NB, P, 4 * C)


def _make_in_maps(x, w_qkv, w_out, b_out, gamma, beta):
    x = np.asarray(x)
    xr = _prep_x(x)
    xtr = _prep_xt(x)
    w_qkv = np.asarray(w_qkv, dtype=np.float32)
    wq = _prep_w(np.ascontiguousarray(w_qkv[0:C].T))
    wk = _prep_w(np.ascontiguousarray(w_qkv[C:2 * C].T))
    # wv in natural orientation: [v-ch, in-ch] chunked along v-ch
    wv = _prep_w(np.ascontiguousarray(w_qkv[2 * C:3 * C]))
    wo = _prep_w(np.ascontiguousarray(np.asarray(w_out, dtype=np.float32).T))
    wall = np.ascontiguousarray(
        np.stack([wq, wk, wv, wo], axis=1)).reshape(P, 4 * KC * C)
    eye = np.eye(P, dtype=np.float16)
    bvec = _prep_vec(np.asarray(b_out, dtype=np.float32))
    gam = _prep_vec(np.asarray(gamma, dtype=np.float32))
    bet = _prep_vec(np.asarray(beta, dtype=np.float32))
    return [
        dict(x=np.ascontiguousarray(xr[c * PB:(c + 1) * PB]),
             xt=np.ascontiguousarray(xtr[c * PB:(c + 1) * PB]),
             wall=wall, eye=eye,
             bvec=bvec, gamma=gam, beta=bet)
        for c in range(NCORES)
    ]


def _run(inputs, trace=False, trace_kwargs=None):
    nc = _get_nc()
    in_maps = _make_in_maps(**inputs)
    res = run_bass_kernel_spmd(nc, in_maps, core_ids=list(range(NCORES)),
                               trace=trace, **(trace_kwargs or {}))
    out = np.concatenate([res.results[c]["out"].astype(np.float32)
                          for c in range(NCORES)], axis=0)
    return out.reshape(B, C, HW_SIDE, HW_SIDE), res


def kernel(x, w_qkv, w_out, b_out, gamma, beta):
    inputs = dict(x=x, w_qkv=w_qkv, w_out=w_out, b_out=b_out,
                  gamma=gamma, beta=beta)
    try:
        out, _ = _run(inputs)
    except Exception:
        # transient device errors (e.g. NRT_EXEC_UNIT_UNRECOVERABLE) have
        # been observed once across many runs; one retry recovers.
        out, _ = _run(inputs)
    return out


# revision 15
# speedup vs baseline: 1.0435x; 1.0089x over previous
"""Trainium2 Bass kernel for nn_MultiHeadAttention_63814624084186.

Reference computation (per batch sample b, fully independent across b):
  x: [512, 4096]  (C channels x N=64*64 pixels)
  qkv = w_qkv @ x            -> q,k,v each [512, 4096] (8 heads x 64 dims)
  scores = (q_h @ k_h^T)/8   -> [64, 64] per head   (channel-attention)
  attn = softmax(scores, -1)
  out_h = attn_h @ v_h       -> [64, 4096]
  y = w_out @ out + b_out    -> [512, 4096]
  y = groupnorm(y over all C,N) * gamma + beta

Sharding: pure data-parallel over batch: 16 samples / 8 cores = 2 per core.

Algebraic restructure — all attention happens in channel space, so the
pixel-sized GEMMs can be collapsed:
  scores_h = q_h k_h^T = wq_h (x x^T) wk_h^T   -> Gram matrix G = x x^T
  y = w_out blockdiag(attn) wv x = W3 x        -> fold W3, never form v
Per-sample PE work: G (upper-tri, 41k cyc) + T1 = G wk^T (8k) + scores
(2k) + W2 fold (2k) + W3 fold (8k) + y = W3 x (66k) ~= 128k cycles vs
~246k for the direct q/k/v formulation.

Design notes:
  - G accumulates in 4 persistent PSUM banks (upper triangle only; the
    lower blocks are PE-transposed from the upper ones afterwards).
  - G/T1/W2/W3 all round to f16; resulting logit error ~0.01 and output
    error ~2e-3, well within tolerance.
  - GroupNorm: bn_stats per PSUM tile (bias folded into the cross-
    partition combine), cross-partition reduce via ones-matmul.
  - DMA descriptor generation (DIRECT2D, ~0.7us per call) serializes on
    the issuing engine, so transfers are split between the sync and
    gpsimd queues and all host-side layouts are per-partition
    contiguous (128 descriptors per DMA).
  - Batch 1's Gram is interleaved into batch 0's output GEMM; batch 0's
    epilogue hides under batch 1's output GEMM.  All 8 score
    accumulations share one PSUM bank so Gram(1) can grab banks the
    moment the G(0) copies drain.
"""

import numpy as np
from contextlib import ExitStack

import concourse.bass as bass
import concourse.tile as tile
from concourse import bacc, mybir
from concourse.bass_utils import run_bass_kernel_spmd

F32 = mybir.dt.float32
F16 = mybir.dt.float16
AX = mybir.AxisListType
ALU = mybir.AluOpType
ACTF = mybir.ActivationFunctionType

B = 16          # global batch
C = 512         # channels
N = 4096        # pixels (64*64)
HW_SIDE = 64
NCORES = 8
PB = B // NCORES  # batches per core
P = 128
KC = C // P     # 4 channel chunks
NB = 8          # pixel blocks of 512
NS = N // 512   # 8 pixel chunks of 512
NHP = 4         # head pairs
EPS = 1e-5


def build_nc():
    nc = bacc.Bacc("TRN2", target_bir_lowering=False, debug=False,
                   num_devices=NCORES)

    xt_d = nc.declare_dram_parameter("xt", [PB, NB, P, 4 * C], F16, isOutput=False)
    x_d = nc.declare_dram_parameter("x", [PB, 2, P, 4 * KC * 512], F16, isOutput=False)
    wall_d = nc.declare_dram_parameter("wall", [P, 4 * KC * C], F16,
                                       isOutput=False)
    eye_d = nc.declare_dram_parameter("eye", [P, P], F16, isOutput=False)
    bias_d = nc.declare_dram_parameter("bvec", [P, KC], F32, isOutput=False)
    gamma_d = nc.declare_dram_parameter("gamma", [P, KC], F32, isOutput=False)
    beta_d = nc.declare_dram_parameter("beta", [P, KC], F32, isOutput=False)
    out_d = nc.declare_dram_parameter("out", [PB, C, N], F16, isOutput=True)

    with tile.TileContext(nc) as tc, ExitStack() as ctx:
        consts = ctx.enter_context(tc.tile_pool(name="consts", bufs=1))
        xtpool = ctx.enter_context(tc.tile_pool(name="xtpool", bufs=6))
        xfpool = ctx.enter_context(tc.tile_pool(name="xfpool", bufs=2))
        gpool = ctx.enter_context(tc.tile_pool(name="gpool", bufs=1))
        t1pool = ctx.enter_context(tc.tile_pool(name="t1pool", bufs=1))
        w2pool = ctx.enter_context(tc.tile_pool(name="w2pool", bufs=1))
        w3pool = ctx.enter_context(tc.tile_pool(name="w3pool", bufs=1))
        ypool = ctx.enter_context(tc.tile_pool(name="ypool", bufs=4))
        attn = ctx.enter_context(tc.tile_pool(name="attn", bufs=2))
        attnt = ctx.enter_context(tc.tile_pool(name="attnt", bufs=4))
        stats = ctx.enter_context(tc.tile_pool(name="stats", bufs=2))
        psg = ctx.enter_context(tc.tile_pool(name="psg", bufs=4, space="PSUM"))
        psy = ctx.enter_context(tc.tile_pool(name="psy", bufs=3, space="PSUM"))
        pssc = ctx.enter_context(tc.tile_pool(name="pssc", bufs=1, space="PSUM"))

        def load_wall():
            t = consts.tile([P, 4, KC, C], F16, tag="wall")
            nc.sync.dma_start(
                out=t, in_=wall_d.ap().rearrange("p (w k c) -> p w k c",
                                                 w=4, k=KC))
            return t[:, 1], t[:, 0], t[:, 2], t[:, 3]  # wk, wq, wv, wo

        bias_sb = consts.tile([P, KC], F32, tag="bias")
        nc.gpsimd.dma_start(out=bias_sb, in_=bias_d[:, :])
        gamma_sb = consts.tile([P, KC], F32, tag="gamma")
        nc.gpsimd.dma_start(out=gamma_sb, in_=gamma_d[:, :])
        beta_sb = consts.tile([P, KC], F32, tag="beta")
        nc.gpsimd.dma_start(out=beta_sb, in_=beta_d[:, :])
        eye_sb = consts.tile([P, P], F16, tag="eye")
        nc.gpsimd.dma_start(out=eye_sb, in_=eye_d[:, :])

        eps_sb = consts.tile([1, 1], F32, tag="eps")
        nc.vector.memset(eps_sb, EPS)
        ones_col = consts.tile([P, 1], F32, tag="ones_col")
        nc.vector.memset(ones_col, 1.0)
        ones_row = consts.tile([1, P], F32, tag="ones_row")
        nc.vector.memset(ones_row, 1.0)

        # per-batch state carried between emission stages
        st_g = {}      # Gram PSUM banks
        st_gsb = {}    # G in SBUF (full, symmetrized)
        st_sc = {}     # score PSUM bank
        st_x = {}      # x [ch, pix] full tile
        st_w3 = {}     # folded W3^T
        st_y = {}
        st_stats = {}
        st_at = {}
        st_scale = {}

        def emit_G_setup(b):
            st_g[b] = [psg.tile([P, C - m * P], F32, tag="psg",
                                name=f"g_{b}_{m}") for m in range(KC)]

        st_xt = {}

        def emit_xt_dma(b, jb, split=False):
            """xt block DMA trigger (sync queue; order = consumption order)."""
            xt = xtpool.tile([P, 4, C], F16, tag="xt", name=f"xt_{b}_{jb}")
            st_xt[(b, jb)] = xt
            if split:
                for t in range(4):
                    nc.sync.dma_start(out=xt[:, t],
                                      in_=xt_d[b, jb, :, t * C:(t + 1) * C])
            else:
                nc.sync.dma_start(
                    out=xt, in_=xt_d[b, jb].rearrange("p (t c) -> p t c", t=4))

        def emit_G_mms(b, jb):
            """upper-tri Gram matmuls for one x^T block (512 pixels)."""
            gps = st_g[b]
            xt = st_xt[(b, jb)]
            for t in range(4):
                pix = jb * 4 + t
                for m in range(KC):
                    nc.tensor.matmul(
                        gps[m],
                        lhsT=xt[:, t, m * P:(m + 1) * P],
                        rhs=xt[:, t, m * P:C],
                        start=(pix == 0), stop=(pix == 31))

        def emit_x_half(b, h):
            """load 4 x [ch, pix] blocks for the output GEMM (contiguous)."""
            if b not in st_x:
                st_x[b] = xfpool.tile([P, NB, KC, 512], F16, tag="xf",
                                      name=f"xf_{b}")
            xf = st_x[b]
            nc.sync.dma_start(
                out=xf[:, 4 * h:4 * (h + 1)],
                in_=x_d[b, h].rearrange("p (j k n) -> p j k n", j=4, k=KC))

        def emit_M_copies(b):
            """PSUM -> SBUF eviction of the Gram upper triangle."""
            gps = st_g[b]
            g_sb = gpool.tile([P, KC, C], F16, tag="gsb", name=f"gsb_{b}")
            st_gsb[b] = g_sb
            for m in range(KC):
                eng = nc.scalar.copy if m % 2 == 0 else nc.vector.tensor_copy
                eng(out=g_sb[:, m, m * P:C], in_=gps[m])

        def emit_M_rest(b):
            """symmetrize G, T1 = G wk^T, per-head scores."""
            g_sb = st_gsb[b]
            # lower blocks via PE transpose of the upper ones
            for m in range(1, KC):
                for k in range(m):
                    pt = psy.tile([P, P], F16, tag="psy", padded_shape=[P, 512])
                    nc.tensor.transpose(pt, g_sb[:, k, m * P:(m + 1) * P],
                                        eye_sb)
                    nc.vector.tensor_copy(out=g_sb[:, m, k * P:(k + 1) * P],
                                          in_=pt)
            t1_sb = t1pool.tile([P, KC, C], F16, tag="t1", name=f"t1_{b}")
            for m in range(KC):
                ps = psy.tile([P, C], F32, tag="psy")
                for k in range(KC):
                    nc.tensor.matmul(
                        ps,
                        lhsT=g_sb[:, k, m * P:(m + 1) * P],
                        rhs=wk_sb[:, k, :],
                        start=(k == 0), stop=(k == KC - 1))
                nc.scalar.copy(out=t1_sb[:, m, :], in_=ps)
            sc_t = pssc.tile([P, NHP, 64], F32, tag="pssc", name=f"sc_{b}")
            st_sc[b] = sc_t
            for hp in range(NHP):
                cl0 = slice(hp * P, hp * P + 64)
                cl1 = slice(hp * P + 64, (hp + 1) * P)
                for k in range(KC):
                    nc.tensor.matmul(
                        sc_t[0:64, hp, :],
                        lhsT=wq_sb[:, k, cl0], rhs=t1_sb[:, k, cl0],
                        start=(k == 0), stop=(k == KC - 1),
                        skip_group_check=True)
                    nc.tensor.matmul(
                        sc_t[64:P, hp, :],
                        lhsT=wq_sb[:, k, cl1], rhs=t1_sb[:, k, cl1],
                        start=(k == 0), stop=(k == KC - 1),
                        skip_group_check=True)

        def emit_softmax(b):
            """softmax on the accumulated score blocks (all pairs batched)."""
            sc_t = st_sc[b]
            mx = attn.tile([P, NHP, 1], F32, tag="mx4")
            nc.vector.reduce_max(out=mx, in_=sc_t, axis=AX.X)
            d_all = attn.tile([P, NHP, 64], F32, tag="d_all")
            nc.vector.tensor_tensor(d_all, sc_t,
                                    mx.to_broadcast([P, NHP, 64]), ALU.subtract)
            e_all = attn.tile([P, NHP, 64], F32, tag="e_all")
            nc.scalar.activation(out=e_all, in_=d_all, func=ACTF.Exp,
                                 bias=0.0, scale=0.125)
            sm = attn.tile([P, NHP, 1], F32, tag="sm4")
            nc.vector.reduce_sum(out=sm, in_=e_all, axis=AX.X)
            rs = attn.tile([P, NHP, 1], F32, tag="rs4")
            nc.vector.reciprocal(out=rs, in_=sm)
            attnT_tiles = []
            for hp in range(NHP):
                at = attnt.tile([P, P], F16, tag="attnT", name=f"at_{b}_{hp}")
                nc.gpsimd.memset(at, 0.0)
                nc.vector.tensor_tensor(
                    at[0:64, 0:64], e_all[0:64, hp, :],
                    rs[0:64, hp, :].to_broadcast([64, 64]), ALU.mult)
                nc.vector.tensor_tensor(
                    at[64:P, 64:P], e_all[64:P, hp, :],
                    rs[64:P, hp, :].to_broadcast([64, 64]), ALU.mult)
                attnT_tiles.append(at)
            st_at[b] = attnT_tiles

        def emit_W3(b):
            """W2 = (blockdiag(A))^T @ woT, then W3^T = wv^T W2^T."""
            attnT_tiles = st_at[b]
            w2 = w2pool.tile([P, KC, C], F16, tag="w2", name=f"w2_{b}")
            for hp in range(NHP):
                at = attnT_tiles[hp]
                ps = psy.tile([P, C], F32, tag="psy")
                nc.tensor.matmul(ps, lhsT=at, rhs=wo_sb[:, hp, :],
                                 start=True, stop=True)
                (nc.scalar.copy if hp % 2 == 0
                 else nc.vector.tensor_copy)(out=w2[:, hp, :], in_=ps)
            w3 = w3pool.tile([P, KC, C], F16, tag="w3", name=f"w3_{b}")
            st_w3[b] = w3
            for m in range(KC):
                ps = psy.tile([P, C], F32, tag="psy")
                for kk in range(KC):
                    nc.tensor.matmul(
                        ps,
                        lhsT=wv_sb[:, kk, m * P:(m + 1) * P],
                        rhs=w2[:, kk, :],
                        start=(kk == 0), stop=(kk == KC - 1))
                (nc.scalar.copy if m % 2 == 0
                 else nc.vector.tensor_copy)(out=w3[:, m, :], in_=ps)

        def emit_By_setup(b):
            y_lo = ypool.tile([P, 2, N], F16, tag="y", name=f"ylo_{b}")
            y_hi = ypool.tile([P, 2, N], F16, tag="y", name=f"yhi_{b}")
            st_y[b] = (y_lo, y_hi)
            st_stats[b] = stats.tile([P, KC, 2], F32, tag="mv",
                                     name=f"mv_{b}")
            st_stats[(b, "raw")] = stats.tile([P, KC, NS, 6], F32,
                                              tag="bnstats", name=f"bst_{b}")

        def emit_By_unit(b, m, ns, pool=None):
            """output GEMM y[m-chunk, ns-block] = W3[m-chunk, :] @ x + stats."""
            w3 = st_w3[b]
            xf = st_x[b]
            y_lo, y_hi = st_y[b]
            st = st_stats[(b, "raw")]
            yt = y_lo if m < 2 else y_hi
            mi = m % 2
            if pool is None:
                ps = psy.tile([P, 512], F32, tag="psy")
            elif pool is psg:
                ps = pool.tile([P, 512], F32, tag="psg", name=f"byg_{b}_{m}_{ns}")
            else:
                ps = pool.tile([P, 512], F32, tag="pssc", name=f"byc_{b}_{m}_{ns}")
            for k in range(KC):
                nc.tensor.matmul(
                    ps,
                    lhsT=w3[:, k, m * P:(m + 1) * P],
                    rhs=xf[:, ns, k, :],
                    start=(k == 0), stop=(k == KC - 1))
            nc.vector.bn_stats(out=st[:, m, ns, :], in_=ps)
            nc.scalar.add(out=yt[:, mi, ns * 512:(ns + 1) * 512],
                          in_=ps, add=bias_sb[:, m:m + 1])
            if ns == NS - 1:
                nc.vector.bn_aggr(out=st_stats[b][:, m, :], in_=st[:, m])

        def emit_tail_stats(b):
            """global mean/var combine."""
            mv = st_stats[b]
            # S[p, stat, m]: 0 = mean+bias, 1 = var, 2 = (mean+bias)^2
            s_t = stats.tile([P, 3, KC], F32, tag="s_t")
            nc.vector.tensor_add(s_t[:, 0, :], mv[:, :, 0], bias_sb)
            nc.vector.tensor_copy(out=s_t[:, 1, :], in_=mv[:, :, 1])
            nc.vector.tensor_mul(s_t[:, 2, :], s_t[:, 0, :], s_t[:, 0, :])
            pstat = psy.tile([1, 3, KC], F32, tag="psy")
            nc.tensor.matmul(pstat, lhsT=ones_col, rhs=s_t,
                             start=True, stop=True)
            red = stats.tile([1, 3], F32, tag="red")
            nc.vector.reduce_sum(out=red, in_=pstat, axis=AX.X)
            e3 = stats.tile([1, 3], F32, tag="e3")
            nc.vector.tensor_scalar_mul(e3, red, 1.0 / C)
            m2 = stats.tile([1, 1], F32, tag="m2")
            nc.vector.tensor_mul(m2, e3[:, 0:1], e3[:, 0:1])
            var = stats.tile([1, 1], F32, tag="var")
            nc.vector.tensor_add(var, e3[:, 1:2], e3[:, 2:3])
            nc.vector.tensor_sub(var, var, m2)
            sc2 = stats.tile([1, 2], F32, tag="sc2")
            nc.vector.tensor_copy(out=sc2[:, 0:1], in_=e3[:, 0:1])
            std = stats.tile([1, 1], F32, tag="std")
            nc.scalar.activation(out=std, in_=var, func=ACTF.Sqrt,
                                 bias=eps_sb, scale=1.0)
            nc.vector.reciprocal(out=sc2[:, 1:2], in_=std)
            bc_ps = psy.tile([P, 2], F32, tag="psy")
            nc.tensor.matmul(bc_ps, lhsT=ones_row, rhs=sc2,
                             start=True, stop=True)
            # s = gamma * rstd ; t = beta - mean_total * s
            s_ch = stats.tile([P, KC], F32, tag="s_ch")
            nc.vector.tensor_scalar_mul(s_ch, gamma_sb, bc_ps[:, 1:2])
            t_ch = stats.tile([P, KC], F32, tag="t_ch")
            nc.vector.tensor_scalar_mul(t_ch, s_ch, bc_ps[:, 0:1])
            nc.vector.tensor_sub(t_ch, beta_sb, t_ch)
            st_scale[b] = (s_ch, t_ch)

        APPLY_SPLIT = 3 * N // 4   # vector is ~3x faster than scalar here

        def emit_apply_slice(b, m, h):
            """normalization apply for one (chunk, section) + writeout."""
            y_lo, y_hi = st_y[b]
            s_ch, t_ch = st_scale[b]
            yt = y_lo if m < 2 else y_hi
            mi = m % 2
            sl = slice(0, APPLY_SPLIT) if h == 0 else slice(APPLY_SPLIT, N)
            if h == 0:
                nc.vector.tensor_scalar(
                    out=yt[:, mi, sl], in0=yt[:, mi, sl],
                    scalar1=s_ch[:, m:m + 1], scalar2=t_ch[:, m:m + 1],
                    op0=ALU.mult, op1=ALU.add)
            else:
                nc.scalar.activation(
                    out=yt[:, mi, sl], in_=yt[:, mi, sl],
                    func=ACTF.Identity,
                    bias=t_ch[:, m:m + 1], scale=s_ch[:, m:m + 1])
            eng = nc.sync if (m + h) % 2 == 0 else nc.gpsimd
            eng.dma_start(out=out_d[b, m * P:(m + 1) * P, sl],
                          in_=yt[:, mi, sl])

        def emit_tail_apply(b):
            for m in range(KC):
                for h in range(2):
                    emit_apply_slice(b, m, h)

        # ---- emission schedule ----
        # sync-queue DMA triggers in exact consumption order: the xt tile
        # slot semaphores pace the whole input stream, and the per-engine
        # descriptor FIFOs then deliver transfers in the same order.
        emit_G_setup(0)
        emit_xt_dma(0, 0, split=True)
        for jb in range(1, NB):
            emit_xt_dma(0, jb)
        wk_sb, wq_sb, wv_sb, wo_sb = load_wall()
        emit_G_setup(1)
        emit_xt_dma(1, 0)
        emit_xt_dma(1, 1)
        emit_x_half(0, 0)
        emit_x_half(0, 1)
        for jb in range(NB):
            emit_G_mms(0, jb)
        emit_M_copies(0)
        emit_M_rest(0)
        emit_xt_dma(1, 2)
        emit_xt_dma(1, 3)
        emit_G_mms(1, 0)
        emit_G_mms(1, 1)
        emit_G_mms(1, 2)
        emit_G_mms(1, 3)
        emit_softmax(0)
        emit_W3(0)
        emit_By_setup(0)
        # interleave batch 0's output GEMM with batch 1's Gram; the last
        # four units are held back to cover batch 1's M-phase latency
        nxt = 4
        u = 0
        held = [(3, ns) for ns in range(4, NS)]
        for m in range(KC):
            for ns in range(NS):
                if (m, ns) in held:
                    continue
                emit_By_unit(0, m, ns,
                             pool=pssc if u % 3 == 2 else None)
                u += 1
                if u % 6 == 0 and nxt < NB:
                    emit_xt_dma(1, nxt)
                    emit_G_mms(1, nxt)
                    nxt += 1
                if u == 18:
                    emit_x_half(1, 0)
                if u == 22:
                    emit_x_half(1, 1)
                if u == 25:
                    emit_M_copies(1)
                if u == 26:
                    emit_M_rest(1)
        emit_softmax(1)
        # pre-warm the Sqrt activation table while the PE is busy
        warm = stats.tile([1, 1], F32, tag="warm")
        nc.scalar.sqrt(out=warm, in_=eps_sb)
        for hu in held:
            emit_By_unit(0, *hu)
        emit_W3(1)
        emit_By_setup(1)
        ap_i = 0
        v = 0
        for m in range(KC):
            for ns in range(NS):
                emit_By_unit(1, m, ns, pool=psg if ns % 2 == 0 else None)
                v += 1
                if v == 2:
                    emit_tail_stats(0)
                if v >= 6 and v % 3 == 0 and ap_i < 2 * KC:
                    emit_apply_slice(0, ap_i // 2, ap_i % 2)
                    ap_i += 1
        emit_tail_stats(1)
        emit_tail_apply(1)

    nc.finalize()
    return nc


_NC_CACHE = {}


def _get_nc():
    if "nc" not in _NC_CACHE:
        _NC_CACHE["nc"] = build_nc()
    return _NC_CACHE["nc"]


def _prep_w(w):
    # [C_in, C_out] -> [128, KC, C_out] fp16 with c_in = k*128 + p
    return np.ascontiguousarray(
        w.reshape(KC, P, C).transpose(1, 0, 2).astype(np.float16))


def _prep_vec(v):
    # [C] -> [128, KC] with c = k*128 + p
    return np.ascontiguousarray(v.reshape(KC, P).T)


def _prep_x(x):
    # [B, C, N] -> [B, 2, P, 4*KC*512] fp16: half h holds pixel blocks
    # 4h..4h+3; per-partition payload (j_local, k, n) is contiguous.
    nb = np.asarray(x).shape[0]
    xr = np.asarray(x, dtype=np.float32).reshape(nb, KC, P, 2, 4, 512)
    return np.ascontiguousarray(
        xr.transpose(0, 3, 2, 4, 1, 5).astype(np.float16)).reshape(
        nb, 2, P, 4 * KC * 512)


def _prep_xt(x):
    # [B, C, N] -> [B, NB, P, 4*C] fp16: xt[b, jb, p, t*C+c] =
    #   x[b, c, (jb*4+t)*128 + p]
    nb = np.asarray(x).shape[0]
    xr = np.asarray(x, dtype=np.float32).reshape(nb, C, NB, 4, P)
    return np.ascontiguousarray(
        xr.transpose(0, 2, 4, 3, 1).astype(np.float16)).reshape(
        nb, NB, P, 4 * C)


def _make_in_maps(x, w_qkv, w_out, b_out, gamma, beta):
    x = np.asarray(x)
    xr = _prep_x(x)
    xtr = _prep_xt(x)
    w_qkv = np.asarray(w_qkv, dtype=np.float32)
    wq = _prep_w(np.ascontiguousarray(w_qkv[0:C].T))
    wk = _prep_w(np.ascontiguousarray(w_qkv[C:2 * C].T))
    # wv in natural orientation: [v-ch, in-ch] chunked along v-ch
    wv = _prep_w(np.ascontiguousarray(w_qkv[2 * C:3 * C]))
    wo = _prep_w(np.ascontiguousarray(np.asarray(w_out, dtype=np.float32).T))
    wall = np.ascontiguousarray(
        np.stack([wq, wk, wv, wo], axis=1)).reshape(P, 4 * KC * C)
    eye = np.eye(P, dtype=np.float16)
    bvec = _prep_vec(np.asarray(b_out, dtype=np.float32))
    gam = _prep_vec(np.asarray(gamma, dtype=np.float32))
    bet = _prep_vec(np.asarray(beta, dtype=np.float32))
    return [
        dict(x=np.ascontiguousarray(xr[c * PB:(c + 1) * PB]),
             xt=np.ascontiguousarray(xtr[c * PB:(c + 1) * PB]),
             wall=wall, eye=eye,
             bvec=bvec, gamma=gam, beta=bet)
        for c in range(NCORES)
    ]


def _run(inputs, trace=False, trace_kwargs=None):
    nc = _get_nc()
    in_maps = _make_in_maps(**inputs)
    res = run_bass_kernel_spmd(nc, in_maps, core_ids=list(range(NCORES)),
                               trace=trace, **(trace_kwargs or {}))
    out = np.concatenate([res.results[c]["out"].astype(np.float32)
                          for c in range(NCORES)], axis=0)
    return out.reshape(B, C, HW_SIDE, HW_SIDE), res


def kernel(x, w_qkv, w_out, b_out, gamma, beta):
    inputs = dict(x=x, w_qkv=w_qkv, w_out=w_out, b_out=b_out,
                  gamma=gamma, beta=beta)
    try:
        out, _ = _run(inputs)
    except Exception:
        # transient device errors (e.g. NRT_EXEC_UNIT_UNRECOVERABLE) have
        # been observed once across many runs; one retry recovers.
        out, _ = _run(inputs)
    return out


# revision 27
# speedup vs baseline: 1.0452x; 1.0017x over previous
"""Trainium2 Bass kernel for nn_MultiHeadAttention_63814624084186.

Reference computation (per batch sample b, fully independent across b):
  x: [512, 4096]  (C channels x N=64*64 pixels)
  qkv = w_qkv @ x            -> q,k,v each [512, 4096] (8 heads x 64 dims)
  scores = (q_h @ k_h^T)/8   -> [64, 64] per head   (channel-attention)
  attn = softmax(scores, -1)
  out_h = attn_h @ v_h       -> [64, 4096]
  y = w_out @ out + b_out    -> [512, 4096]
  y = groupnorm(y over all C,N) * gamma + beta

Sharding: pure data-parallel over batch: 16 samples / 8 cores = 2 per core.

Algebraic restructure — all attention happens in channel space, so the
pixel-sized GEMMs can be collapsed:
  scores_h = q_h k_h^T = wq_h (x x^T) wk_h^T   -> Gram matrix G = x x^T
  y = w_out blockdiag(attn) wv x = W3 x        -> fold W3, never form v
Per-sample PE work: G (upper-tri, 41k cyc) + T1 = G wk^T (8k) + scores
(2k) + W2 fold (2k) + W3 fold (8k) + y = W3 x (66k) ~= 128k cycles vs
~246k for the direct q/k/v formulation.

Design notes:
  - G accumulates in 4 persistent PSUM banks (upper triangle only; the
    lower blocks are PE-transposed from the upper ones afterwards).
  - G/T1/W2/W3 all round to f16; resulting logit error ~0.01 and output
    error ~2e-3, well within tolerance.
  - GroupNorm: bn_stats per PSUM tile (bias folded into the cross-
    partition combine), cross-partition reduce via ones-matmul.
  - DMA descriptor generation (DIRECT2D, ~0.7us per call) serializes on
    the issuing engine, so transfers are split between the sync and
    gpsimd queues and all host-side layouts are per-partition
    contiguous (128 descriptors per DMA).
  - Batch 1's Gram is interleaved into batch 0's output GEMM; batch 0's
    epilogue hides under batch 1's output GEMM.  All 8 score
    accumulations share one PSUM bank so Gram(1) can grab banks the
    moment the G(0) copies drain.
"""

import numpy as np
from contextlib import ExitStack

import concourse.bass as bass
import concourse.tile as tile
from concourse import bacc, mybir
from concourse.bass_utils import run_bass_kernel_spmd

F32 = mybir.dt.float32
F16 = mybir.dt.float16
AX = mybir.AxisListType
ALU = mybir.AluOpType
ACTF = mybir.ActivationFunctionType

B = 16          # global batch
C = 512         # channels
N = 4096        # pixels (64*64)
HW_SIDE = 64
NCORES = 8
PB = B // NCORES  # batches per core
P = 128
KC = C // P     # 4 channel chunks
NB = 8          # pixel blocks of 512
NS = N // 512   # 8 pixel chunks of 512
NHP = 4         # head pairs
EPS = 1e-5


def build_nc():
    nc = bacc.Bacc("TRN2", target_bir_lowering=False, debug=False,
                   num_devices=NCORES)

    xt_d = nc.declare_dram_parameter("xt", [PB, NB, P, 4 * C], F16, isOutput=False)
    x_d = nc.declare_dram_parameter("x", [PB, 2, P, 4 * KC * 512], F16, isOutput=False)
    wall_d = nc.declare_dram_parameter("wall", [P, 4 * KC * C], F16,
                                       isOutput=False)
    eye_d = nc.declare_dram_parameter("eye", [P, P], F16, isOutput=False)
    bias_d = nc.declare_dram_parameter("bvec", [P, KC], F32, isOutput=False)
    gamma_d = nc.declare_dram_parameter("gamma", [P, KC], F32, isOutput=False)
    beta_d = nc.declare_dram_parameter("beta", [P, KC], F32, isOutput=False)
    out_d = nc.declare_dram_parameter("out", [PB, C, N], F16, isOutput=True)

    with tile.TileContext(nc) as tc, ExitStack() as ctx:
        consts = ctx.enter_context(tc.tile_pool(name="consts", bufs=1))
        xtpool = ctx.enter_context(tc.tile_pool(name="xtpool", bufs=6))
        xfpool = ctx.enter_context(tc.tile_pool(name="xfpool", bufs=2))
        gpool = ctx.enter_context(tc.tile_pool(name="gpool", bufs=1))
        t1pool = ctx.enter_context(tc.tile_pool(name="t1pool", bufs=1))
        w2pool = ctx.enter_context(tc.tile_pool(name="w2pool", bufs=1))
        w3pool = ctx.enter_context(tc.tile_pool(name="w3pool", bufs=1))
        ypool = ctx.enter_context(tc.tile_pool(name="ypool", bufs=4))
        attn = ctx.enter_context(tc.tile_pool(name="attn", bufs=2))
        attnt = ctx.enter_context(tc.tile_pool(name="attnt", bufs=4))
        stats = ctx.enter_context(tc.tile_pool(name="stats", bufs=2))
        psg = ctx.enter_context(tc.tile_pool(name="psg", bufs=4, space="PSUM"))
        psy = ctx.enter_context(tc.tile_pool(name="psy", bufs=3, space="PSUM"))
        pssc = ctx.enter_context(tc.tile_pool(name="pssc", bufs=1, space="PSUM"))

        def load_wall():
            t = consts.tile([P, 4, KC, C], F16, tag="wall")
            nc.sync.dma_start(
                out=t, in_=wall_d.ap().rearrange("p (w k c) -> p w k c",
                                                 w=4, k=KC))
            return t[:, 1], t[:, 0], t[:, 2], t[:, 3]  # wk, wq, wv, wo

        bias_sb = consts.tile([P, KC], F32, tag="bias")
        nc.gpsimd.dma_start(out=bias_sb, in_=bias_d[:, :])
        gamma_sb = consts.tile([P, KC], F32, tag="gamma")
        nc.gpsimd.dma_start(out=gamma_sb, in_=gamma_d[:, :])
        beta_sb = consts.tile([P, KC], F32, tag="beta")
        nc.gpsimd.dma_start(out=beta_sb, in_=beta_d[:, :])
        eye_sb = consts.tile([P, P], F16, tag="eye")
        nc.gpsimd.dma_start(out=eye_sb, in_=eye_d[:, :])

        eps_sb = consts.tile([1, 1], F32, tag="eps")
        nc.vector.memset(eps_sb, EPS)
        ones_col = consts.tile([P, 1], F32, tag="ones_col")
        nc.vector.memset(ones_col, 1.0)
        ones_row = consts.tile([1, P], F32, tag="ones_row")
        nc.vector.memset(ones_row, 1.0)

        # per-batch state carried between emission stages
        st_g = {}      # Gram PSUM banks
        st_gsb = {}    # G in SBUF (full, symmetrized)
        st_sc = {}     # score PSUM bank
        st_x = {}      # x [ch, pix] full tile
        st_w3 = {}     # folded W3^T
        st_y = {}
        st_stats = {}
        st_at = {}
        st_scale = {}

        def emit_G_setup(b):
            st_g[b] = [psg.tile([P, C - m * P], F32, tag="psg",
                                name=f"g_{b}_{m}") for m in range(KC)]

        st_xt = {}

        def emit_xt_dma(b, jb, split=False):
            """xt block DMA trigger (sync queue; order = consumption order)."""
            xt = xtpool.tile([P, 4, C], F16, tag="xt", name=f"xt_{b}_{jb}")
            st_xt[(b, jb)] = xt
            if split:
                for t in range(4):
                    nc.sync.dma_start(out=xt[:, t],
                                      in_=xt_d[b, jb, :, t * C:(t + 1) * C])
            else:
                nc.sync.dma_start(
                    out=xt, in_=xt_d[b, jb].rearrange("p (t c) -> p t c", t=4))

        def emit_G_mms(b, jb):
            """upper-tri Gram matmuls for one x^T block (512 pixels)."""
            gps = st_g[b]
            xt = st_xt[(b, jb)]
            for t in range(4):
                pix = jb * 4 + t
                for m in range(KC):
                    nc.tensor.matmul(
                        gps[m],
                        lhsT=xt[:, t, m * P:(m + 1) * P],
                        rhs=xt[:, t, m * P:C],
                        start=(pix == 0), stop=(pix == 31))

        def emit_x_half(b, h):
            """load 4 x [ch, pix] blocks for the output GEMM (contiguous)."""
            if b not in st_x:
                st_x[b] = xfpool.tile([P, NB, KC, 512], F16, tag="xf",
                                      name=f"xf_{b}")
            xf = st_x[b]
            nc.sync.dma_start(
                out=xf[:, 4 * h:4 * (h + 1)],
                in_=x_d[b, h].rearrange("p (j k n) -> p j k n", j=4, k=KC))

        def emit_M_copies(b):
            """PSUM -> SBUF eviction of the Gram upper triangle."""
            gps = st_g[b]
            g_sb = gpool.tile([P, KC, C], F16, tag="gsb", name=f"gsb_{b}")
            st_gsb[b] = g_sb
            for m in range(KC):
                eng = nc.scalar.copy if m % 2 == 0 else nc.vector.tensor_copy
                eng(out=g_sb[:, m, m * P:C], in_=gps[m])

        def emit_M_rest(b):
            """symmetrize G, T1 = G wk^T, per-head scores."""
            g_sb = st_gsb[b]
            # lower blocks via PE transpose of the upper ones
            for m in range(1, KC):
                for k in range(m):
                    pt = psy.tile([P, P], F16, tag="psy", padded_shape=[P, 512])
                    nc.tensor.transpose(pt, g_sb[:, k, m * P:(m + 1) * P],
                                        eye_sb)
                    nc.vector.tensor_copy(out=g_sb[:, m, k * P:(k + 1) * P],
                                          in_=pt)
            t1_sb = t1pool.tile([P, KC, C], F16, tag="t1", name=f"t1_{b}")
            for m in range(KC):
                ps = psy.tile([P, C], F32, tag="psy")
                for k in range(KC):
                    nc.tensor.matmul(
                        ps,
                        lhsT=g_sb[:, k, m * P:(m + 1) * P],
                        rhs=wk_sb[:, k, :],
                        start=(k == 0), stop=(k == KC - 1))
                nc.scalar.copy(out=t1_sb[:, m, :], in_=ps)
            sc_t = pssc.tile([P, NHP, 64], F32, tag="pssc", name=f"sc_{b}")
            st_sc[b] = sc_t
            for hp in range(NHP):
                cl0 = slice(hp * P, hp * P + 64)
                cl1 = slice(hp * P + 64, (hp + 1) * P)
                for k in range(KC):
                    nc.tensor.matmul(
                        sc_t[0:64, hp, :],
                        lhsT=wq_sb[:, k, cl0], rhs=t1_sb[:, k, cl0],
                        start=(k == 0), stop=(k == KC - 1),
                        skip_group_check=True)
                    nc.tensor.matmul(
                        sc_t[64:P, hp, :],
                        lhsT=wq_sb[:, k, cl1], rhs=t1_sb[:, k, cl1],
                        start=(k == 0), stop=(k == KC - 1),
                        skip_group_check=True)

        def emit_softmax(b):
            """softmax on the accumulated score blocks (all pairs batched)."""
            sc_t = st_sc[b]
            mx = attn.tile([P, NHP, 1], F32, tag="mx4")
            nc.vector.reduce_max(out=mx, in_=sc_t, axis=AX.X)
            d_all = attn.tile([P, NHP, 64], F32, tag="d_all")
            nc.vector.tensor_tensor(d_all, sc_t,
                                    mx.to_broadcast([P, NHP, 64]), ALU.subtract)
            e_all = attn.tile([P, NHP, 64], F32, tag="e_all")
            nc.scalar.activation(out=e_all, in_=d_all, func=ACTF.Exp,
                                 bias=0.0, scale=0.125)
            sm = attn.tile([P, NHP, 1], F32, tag="sm4")
            nc.vector.reduce_sum(out=sm, in_=e_all, axis=AX.X)
            rs = attn.tile([P, NHP, 1], F32, tag="rs4")
            nc.vector.reciprocal(out=rs, in_=sm)
            attnT_tiles = []
            for hp in range(NHP):
                at = attnt.tile([P, P], F16, tag="attnT", name=f"at_{b}_{hp}")
                nc.gpsimd.memset(at, 0.0)
                nc.vector.tensor_tensor(
                    at[0:64, 0:64], e_all[0:64, hp, :],
                    rs[0:64, hp, :].to_broadcast([64, 64]), ALU.mult)
                nc.vector.tensor_tensor(
                    at[64:P, 64:P], e_all[64:P, hp, :],
                    rs[64:P, hp, :].to_broadcast([64, 64]), ALU.mult)
                attnT_tiles.append(at)
            st_at[b] = attnT_tiles

        def emit_W3(b):
            """W2 = (blockdiag(A))^T @ woT, then W3^T = wv^T W2^T."""
            attnT_tiles = st_at[b]
            w2 = w2pool.tile([P, KC, C], F16, tag="w2", name=f"w2_{b}")
            for hp in range(NHP):
                at = attnT_tiles[hp]
                ps = psy.tile([P, C], F32, tag="psy")
                nc.tensor.matmul(ps, lhsT=at, rhs=wo_sb[:, hp, :],
                                 start=True, stop=True)
                (nc.scalar.copy if hp % 2 == 0
                 else nc.vector.tensor_copy)(out=w2[:, hp, :], in_=ps)
            w3 = w3pool.tile([P, KC, C], F16, tag="w3", name=f"w3_{b}")
            st_w3[b] = w3
            for m in range(KC):
                ps = psy.tile([P, C], F32, tag="psy")
                for kk in range(KC):
                    nc.tensor.matmul(
                        ps,
                        lhsT=wv_sb[:, kk, m * P:(m + 1) * P],
                        rhs=w2[:, kk, :],
                        start=(kk == 0), stop=(kk == KC - 1))
                (nc.scalar.copy if m % 2 == 0
                 else nc.vector.tensor_copy)(out=w3[:, m, :], in_=ps)

        def emit_By_setup(b):
            y_lo = ypool.tile([P, 2, N], F16, tag="y", name=f"ylo_{b}")
            y_hi = ypool.tile([P, 2, N], F16, tag="y", name=f"yhi_{b}")
            st_y[b] = (y_lo, y_hi)
            st_stats[b] = stats.tile([P, KC, 2], F32, tag="mv",
                                     name=f"mv_{b}")
            st_stats[(b, "raw")] = stats.tile([P, KC, NS, 6], F32,
                                              tag="bnstats", name=f"bst_{b}")

        def emit_By_unit(b, m, ns, pool=None):
            """output GEMM y[m-chunk, ns-block] = W3[m-chunk, :] @ x + stats."""
            w3 = st_w3[b]
            xf = st_x[b]
            y_lo, y_hi = st_y[b]
            st = st_stats[(b, "raw")]
            yt = y_lo if m < 2 else y_hi
            mi = m % 2
            if pool is None:
                ps = psy.tile([P, 512], F32, tag="psy")
            elif pool is psg:
                ps = pool.tile([P, 512], F32, tag="psg", name=f"byg_{b}_{m}_{ns}")
            else:
                ps = pool.tile([P, 512], F32, tag="pssc", name=f"byc_{b}_{m}_{ns}")
            for k in range(KC):
                nc.tensor.matmul(
                    ps,
                    lhsT=w3[:, k, m * P:(m + 1) * P],
                    rhs=xf[:, ns, k, :],
                    start=(k == 0), stop=(k == KC - 1))
            nc.vector.bn_stats(out=st[:, m, ns, :], in_=ps)
            nc.scalar.add(out=yt[:, mi, ns * 512:(ns + 1) * 512],
                          in_=ps, add=bias_sb[:, m:m + 1])
            if ns == NS - 1:
                nc.vector.bn_aggr(out=st_stats[b][:, m, :], in_=st[:, m])

        def emit_tail_stats(b):
            """global mean/var combine."""
            mv = st_stats[b]
            # S[p, stat, m]: 0 = mean+bias, 1 = var, 2 = (mean+bias)^2
            s_t = stats.tile([P, 3, KC], F32, tag="s_t")
            nc.vector.tensor_add(s_t[:, 0, :], mv[:, :, 0], bias_sb)
            nc.vector.tensor_copy(out=s_t[:, 1, :], in_=mv[:, :, 1])
            nc.vector.tensor_mul(s_t[:, 2, :], s_t[:, 0, :], s_t[:, 0, :])
            pstat = psy.tile([1, 3, KC], F32, tag="psy")
            nc.tensor.matmul(pstat, lhsT=ones_col, rhs=s_t,
                             start=True, stop=True)
            red = stats.tile([1, 3], F32, tag="red")
            nc.vector.reduce_sum(out=red, in_=pstat, axis=AX.X)
            e3 = stats.tile([1, 3], F32, tag="e3")
            nc.vector.tensor_scalar_mul(e3, red, 1.0 / C)
            m2 = stats.tile([1, 1], F32, tag="m2")
            nc.vector.tensor_mul(m2, e3[:, 0:1], e3[:, 0:1])
            var = stats.tile([1, 1], F32, tag="var")
            nc.vector.tensor_add(var, e3[:, 1:2], e3[:, 2:3])
            nc.vector.tensor_sub(var, var, m2)
            sc2 = stats.tile([1, 2], F32, tag="sc2")
            nc.vector.tensor_copy(out=sc2[:, 0:1], in_=e3[:, 0:1])
            std = stats.tile([1, 1], F32, tag="std")
            nc.scalar.activation(out=std, in_=var, func=ACTF.Sqrt,
                                 bias=eps_sb, scale=1.0)
            nc.vector.reciprocal(out=sc2[:, 1:2], in_=std)
            bc_ps = psy.tile([P, 2], F32, tag="psy")
            nc.tensor.matmul(bc_ps, lhsT=ones_row, rhs=sc2,
                             start=True, stop=True)
            # s = gamma * rstd ; t = beta - mean_total * s
            s_ch = stats.tile([P, KC], F32, tag="s_ch")
            nc.vector.tensor_scalar_mul(s_ch, gamma_sb, bc_ps[:, 1:2])
            t_ch = stats.tile([P, KC], F32, tag="t_ch")
            nc.vector.tensor_scalar_mul(t_ch, s_ch, bc_ps[:, 0:1])
            nc.vector.tensor_sub(t_ch, beta_sb, t_ch)
            st_scale[b] = (s_ch, t_ch)

        APPLY_SPLIT = 3 * N // 4   # vector is ~3x faster than scalar here

        def emit_apply_slice(b, m, h):
            """normalization apply for one (chunk, section) + writeout."""
            y_lo, y_hi = st_y[b]
            s_ch, t_ch = st_scale[b]
            yt = y_lo if m < 2 else y_hi
            mi = m % 2
            sl = slice(0, APPLY_SPLIT) if h == 0 else slice(APPLY_SPLIT, N)
            if h == 0:
                nc.vector.tensor_scalar(
                    out=yt[:, mi, sl], in0=yt[:, mi, sl],
                    scalar1=s_ch[:, m:m + 1], scalar2=t_ch[:, m:m + 1],
                    op0=ALU.mult, op1=ALU.add)
            else:
                nc.scalar.activation(
                    out=yt[:, mi, sl], in_=yt[:, mi, sl],
                    func=ACTF.Identity,
                    bias=t_ch[:, m:m + 1], scale=s_ch[:, m:m + 1])
            eng = nc.sync if (m + h) % 2 == 0 else nc.gpsimd
            eng.dma_start(out=out_d[b, m * P:(m + 1) * P, sl],
                          in_=yt[:, mi, sl])

        def emit_tail_apply(b):
            for m in range(KC):
                for h in range(2):
                    emit_apply_slice(b, m, h)

        # ---- emission schedule ----
        # sync-queue DMA triggers in exact consumption order: the xt tile
        # slot semaphores pace the whole input stream, and the per-engine
        # descriptor FIFOs then deliver transfers in the same order.
        emit_G_setup(0)
        emit_xt_dma(0, 0, split=True)
        for jb in range(1, NB):
            emit_xt_dma(0, jb)
        wk_sb, wq_sb, wv_sb, wo_sb = load_wall()
        emit_G_setup(1)
        emit_xt_dma(1, 0)
        emit_xt_dma(1, 1)
        emit_x_half(0, 0)
        emit_x_half(0, 1)
        for jb in range(NB):
            emit_G_mms(0, jb)
        emit_M_copies(0)
        emit_M_rest(0)
        emit_xt_dma(1, 2)
        emit_xt_dma(1, 3)
        emit_G_mms(1, 0)
        emit_G_mms(1, 1)
        emit_G_mms(1, 2)
        emit_G_mms(1, 3)
        emit_softmax(0)
        emit_W3(0)
        emit_By_setup(0)
        # interleave batch 0's output GEMM with batch 1's Gram; the last
        # four units are held back to cover batch 1's M-phase latency
        nxt = 4
        u = 0
        held = [(3, ns) for ns in range(4, NS)]
        for m in range(KC):
            for ns in range(NS):
                if (m, ns) in held:
                    continue
                emit_By_unit(0, m, ns,
                             pool=pssc if u % 3 == 2 else None)
                u += 1
                if u % 6 == 0 and nxt < NB:
                    emit_xt_dma(1, nxt)
                    emit_G_mms(1, nxt)
                    nxt += 1
                if u == 18:
                    emit_x_half(1, 0)
                if u == 22:
                    emit_x_half(1, 1)
                if u == 25:
                    emit_M_copies(1)
                if u == 26:
                    emit_M_rest(1)
        emit_softmax(1)
        # pre-warm the Sqrt activation table while the PE is busy
        warm = stats.tile([1, 1], F32, tag="warm")
        nc.scalar.sqrt(out=warm, in_=eps_sb)
        for hu in held:
            emit_By_unit(0, *hu)
        emit_W3(1)
        emit_By_setup(1)
        ap_i = 0
        v = 0
        for m in range(KC):
            for ns in range(NS):
                emit_By_unit(1, m, ns, pool=psg if ns % 2 == 0 else None)
                v += 1
                if v == 2:
                    emit_tail_stats(0)
                if v >= 6 and v % 3 == 0 and ap_i < 2 * KC:
                    emit_apply_slice(0, ap_i // 2, ap_i % 2)
                    ap_i += 1
        emit_tail_stats(1)
        emit_tail_apply(1)

    nc.finalize()
    return nc


_NC_CACHE = {}


def _get_nc():
    if "nc" not in _NC_CACHE:
        _NC_CACHE["nc"] = build_nc()
    return _NC_CACHE["nc"]


def _prep_w(w):
    # [C_in, C_out] -> [128, KC, C_out] fp16 with c_in = k*128 + p
    return np.ascontiguousarray(
        w.reshape(KC, P, C).transpose(1, 0, 2).astype(np.float16))


def _prep_vec(v):
    # [C] -> [128, KC] with c = k*128 + p
    return np.ascontiguousarray(v.reshape(KC, P).T)


def _prep_x(x):
    # [B, C, N] -> [B, 2, P, 4*KC*512] fp16: half h holds pixel blocks
    # 4h..4h+3; per-partition payload (j_local, k, n) is contiguous.
    nb = np.asarray(x).shape[0]
    xr = np.asarray(x, dtype=np.float32).reshape(nb, KC, P, 2, 4, 512)
    return np.ascontiguousarray(
        xr.transpose(0, 3, 2, 4, 1, 5).astype(np.float16)).reshape(
        nb, 2, P, 4 * KC * 512)


def _prep_xt(x):
    # [B, C, N] -> [B, NB, P, 4*C] fp16: xt[b, jb, p, t*C+c] =
    #   x[b, c, (jb*4+t)*128 + p]
    nb = np.asarray(x).shape[0]
    xr = np.asarray(x, dtype=np.float32).reshape(nb, C, NB, 4, P)
    return np.ascontiguousarray(
        xr.transpose(0, 2, 4, 3, 1).astype(np.float16)).reshape(
        nb, NB, P, 4 * C)


def _make_in_maps(x, w_qkv, w_out, b_out, gamma, beta):
    x = np.asarray(x)
    xr = _prep_x(x)
    xtr = _prep_xt(x)
    w_qkv = np.asarray(w_qkv, dtype=np.float32)
    wq = _prep_w(np.ascontiguousarray(w_qkv[0:C].T))
    wk = _prep_w(np.ascontiguousarray(w_qkv[C:2 * C].T))
    # wv in natural orientation: [v-ch, in-ch] chunked along v-ch
    wv = _prep_w(np.ascontiguousarray(w_qkv[2 * C:3 * C]))
    wo = _prep_w(np.ascontiguousarray(np.asarray(w_out, dtype=np.float32).T))
    wall = np.ascontiguousarray(
        np.stack([wq, wk, wv, wo], axis=1)).reshape(P, 4 * KC * C)
    eye = np.eye(P, dtype=np.float16)
    bvec = _prep_vec(np.asarray(b_out, dtype=np.float32))
    gam = _prep_vec(np.asarray(gamma, dtype=np.float32))
    bet = _prep_vec(np.asarray(beta, dtype=np.float32))
    return [
        dict(x=np.ascontiguousarray(xr[c * PB:(c + 1) * PB]),
             xt=np.ascontiguousarray(xtr[c * PB:(c + 1) * PB]),
             wall=wall, eye=eye,
             bvec=bvec, gamma=gam, beta=bet)
        for c in range(NCORES)
    ]


def _run(inputs, trace=False, trace_kwargs=None):
    nc = _get_nc()
    in_maps = _make_in_maps(**inputs)
    res = run_bass_kernel_spmd(nc, in_maps, core_ids=list(range(NCORES)),
                               trace=trace, **(trace_kwargs or {}))
    out = np.concatenate([res.results[c]["out"].astype(np.float32)
                          for c in range(NCORES)], axis=0)
    return out.reshape(B, C, HW_SIDE, HW_SIDE), res


def kernel(x, w_qkv, w_out, b_out, gamma, beta):
    inputs = dict(x=x, w_qkv=w_qkv, w_out=w_out, b_out=b_out,
                  gamma=gamma, beta=beta)
    try:
        out, _ = _run(inputs)
    except Exception:
        # transient device errors (e.g. NRT_EXEC_UNIT_UNRECOVERABLE) have
        # been observed once across many runs; one retry recovers.
        out, _ = _run(inputs)
    return out
